# revision 3
# baseline (speedup 1.0000x reference)
"""Fused multi-head self-attention kernel for Trainium2 (Bass/Tile). v2.

Problem: x:[4,2560,320] f32, Wq/Wk/Wv:[320,512], Wo:[512,320], bo:[320]
  q,k,v = x@W*, 8 heads x 64; sim = q k^T * d^-0.5; attn = softmax(sim);
  out = (attn @ v) @ Wo + bo.

Sharding: batch*head 32-way -> 8 cores: core c handles batch c//2 and the
4-head group c%2. Host sums the two half-head partial output projections
per batch and adds the bias.

Per-core layout trick: scores are computed TRANSPOSED (sT[j,i] = k_j . q_i)
so that the softmax denominator arrives for free: v gets a ones-column
appended, and out' = expT_slice.T @ [v|1] accumulates both attn@v and the
row sums. Normalisation is a per-partition reciprocal+multiply.

Engine budget (cost model, per core): PE ~160us is the floor (scores
204.8k cyc + attn@v 104k + projections/transposes ~69k). The ACT-engine
exp over 26M elements (~218us alone -- the old bottleneck) is split:
 - ACT computes exp on score cols [0:ACOLS] natively; DVE computes cols
   [ACOLS:1280] with a one-instruction Schraudolph bitcast exp
   (int16(x*A+B) viewed as bf16; ~1.8% rms there, 40% of weights).
 - eta/etb live in SEPARATE tiles and the scores land in two psum tiles
   (s1 ACT-read / s2 DVE-read), so neither engine's slot chain gates the
   other (shared tiles serialize via write-write/read hazards).
 - x/Wq/Wk/Wv/Wo are bf16 (host-side cast): halves DMA + SBUF footprint.
 - attn normalize ('on') is bf16 so PE transposes run at 1 cyc/row; outT
   is bf16; transposes flushed in pairs; attn transpose+flush deferred one
   step so the in-order PE queue never waits on the DVE normalize.
 - all y-steps run in the tail (keeps the last block's ACT stream
   exp-only); tail attn runs i>=6 first (needs only DVE-produced etb).
"""

import sys

import numpy as np

if "/opt/trn_rl_repo" not in sys.path:
    sys.path.insert(0, "/opt/trn_rl_repo")

from contextlib import ExitStack

import concourse.bass as bass
from concourse import bacc
import concourse.mybir as mybir
import concourse.tile as tile
from concourse.bass_utils import run_bass_kernel_spmd
from concourse.masks import make_identity

# ---- problem constants (hardcoded per contract) ----
B = 4
N = 2560
QD = 320
H_TOT = 8
D = 64
HPC = 4                  # heads per core
IPC = HPC * D            # 256 inner dims per core
SCALE = D ** -0.5
NT = N // 128            # 20 n-tiles
HALF = N // 2            # 1280
F32 = mybir.dt.float32
BF16 = mybir.dt.bfloat16
F32R = mybir.dt.float32r
I16 = mybir.dt.int16
EXP = mybir.ActivationFunctionType.Exp

# qd (=320) split into K subtiles for the 128-partition contraction
KS = [(0, 128), (128, 128), (256, 64)]
# 1280-wide column chunks (PSUM-bank-aligned matmul N<=512)
CHUNKS = [(0, 512), (512, 512), (1024, 256)]

# exp split: ACT native exp on [0:ACOLS], DVE Schraudolph on [ACOLS:HALF].
# ACOLS must be a multiple of 128 so the two exp outputs can live in
# SEPARATE tiles (shared-tile writes serialize ACT behind DVE via a
# write-write hazard) with attnv i-tiles reading wholly from one of them.
ACOLS = 768
A_MUL = 128.0 * SCALE / np.log(2.0)       # folds the 1/sqrt(d) scale
B_ADD = 128.0 * (127.0 - 0.058)           # rounding-convert-optimal bias

EXP_BUFS = 43            # expT pool slots of [128,1280] bf16 (2.5KB/part each)

_built = {}
last_results = None      # stashed BassKernelResults for the test harness


def _build():
    nc = bacc.Bacc(None, target_bir_lowering=False)
    xT = nc.declare_dram_parameter("xT", [QD, N], BF16, isOutput=False)
    wq = nc.declare_dram_parameter("wq", [QD, IPC], BF16, isOutput=False)
    wk = nc.declare_dram_parameter("wk", [QD, IPC], BF16, isOutput=False)
    wv = nc.declare_dram_parameter("wv", [QD, IPC], BF16, isOutput=False)
    wo = nc.declare_dram_parameter("wo", [IPC, QD], BF16, isOutput=False)
    y = nc.declare_dram_parameter("y", [N, QD], F32, isOutput=True)

    with tile.TileContext(nc) as tc, ExitStack() as ctx:
        const = ctx.enter_context(tc.tile_pool(name="const", bufs=1))
        smps = ctx.enter_context(tc.tile_pool(name="smps", bufs=2, space="PSUM"))
        epool = ctx.enter_context(tc.tile_pool(name="epool", bufs=EXP_BUFS))
        sbsm = ctx.enter_context(tc.tile_pool(name="sbsm", bufs=4))
        ypool = ctx.enter_context(tc.tile_pool(name="ypool", bufs=5))
        spool_cm = tc.tile_pool(name="spool", bufs=2, space="PSUM")
        spool = spool_cm.__enter__()

        ident = const.tile([128, 128], F32, tag="ident", name="ident")
        make_identity(nc, ident[:])
        identb = const.tile([128, 128], BF16, tag="identb", name="identb")
        nc.vector.tensor_copy(identb[:], ident[:])
        warm = sbsm.tile([128, 1], F32, tag="rc", name="warm")
        nc.scalar.activation(warm[:], ident[:, 0:1], EXP, scale=1.0)
        for _ in range(6):
            pw = smps.tile([128, 128], F32, tag="sm", name="pwarm")
            nc.tensor.matmul(pw[:], lhsT=ident[:], rhs=ident[:],
                             start=True, stop=True)

        # ---- persistent inputs (DMA emission ordered by first use) ----
        xts = [const.tile([128, N], BF16, tag=f"xt{ki}", name=f"xt{ki}")
               for ki in range(3)]
        wqs = [const.tile([128, IPC], BF16, tag=f"wq{ki}", name=f"wq{ki}")
               for ki in range(3)]
        wks = [const.tile([128, IPC], BF16, tag=f"wk{ki}", name=f"wk{ki}")
               for ki in range(3)]
        wvs = [const.tile([128, IPC], BF16, tag=f"wv{ki}", name=f"wv{ki}")
               for ki in range(3)]
        wos = [const.tile([128, QD], BF16, tag=f"wo{kk}", name=f"wo{kk}")
               for kk in range(2)]
        # critical set first; x split across BOTH dma queues (sync + gpsimd)
        for ki, (k0, kw) in enumerate(KS):
            nc.sync.dma_start(xts[ki][:kw, 0:640], xT[k0:k0 + kw, 0:640])
            nc.gpsimd.dma_start(wqs[ki][:kw, :], wq[k0:k0 + kw, :])
        for ki, (k0, kw) in enumerate(KS):
            nc.sync.dma_start(xts[ki][:kw, 640:1280], xT[k0:k0 + kw, 640:1280])
            nc.gpsimd.dma_start(wks[ki][:kw, :], wk[k0:k0 + kw, :])
        for ki, (k0, kw) in enumerate(KS):
            nc.gpsimd.dma_start(xts[ki][:kw, 1280:1920], xT[k0:k0 + kw, 1280:1920])
            nc.sync.dma_start(xts[ki][:kw, 1920:2560], xT[k0:k0 + kw, 1920:2560])
        for ki, (k0, kw) in enumerate(KS):
            nc.gpsimd.dma_start(wvs[ki][:kw, :], wv[k0:k0 + kw, :])
        for kk in range(2):
            nc.gpsimd.dma_start(wos[kk][:], wo[kk * 128:(kk + 1) * 128, :])

        # qT/kT: [inner(256) x n] as 2 tiles of [128, N] each; fp32 storage
        qk_sb = [const.tile([128, N], F32R, tag=f"qk{i}", name=f"qk{i}") for i in range(4)]
        # outT: normalized attention output, [inner x n], bf16
        outT = [const.tile([128, N], BF16, tag=f"oT{kk}", name=f"oT{kk}") for kk in range(2)]
        # v with ones column per head: [n-tile][128, 4*65] bf16
        v1s = [const.tile([128, HPC * 65], BF16, tag=f"v1_{j}", name=f"v1_{j}") for j in range(NT)]

        ws = [wqs, wks]
        tails = {}
        tstate = {}

        def qk_proj(ti, m, half, chunks=None):
            """qT/kT tile ti(0=q,1=k), inner slab m, col half -> qk_sb[ti*2+m].

            PSUM->SBUF copies: 512-wide chunks go to ACT (has startup slack),
            the 256-wide chunk to DVE, to keep early-block DVE load down.
            """
            for c0, cw in (chunks or CHUNKS):
                ps = smps.tile([128, 512], F32, tag="sm", name="smp")
                for ki, (k0, kw) in enumerate(KS):
                    nc.tensor.matmul(
                        ps[:, 0:cw],
                        lhsT=ws[ti][ki][:kw, m * 128:(m + 1) * 128],
                        rhs=xts[ki][:kw, half * HALF + c0:half * HALF + c0 + cw],
                        start=(ki == 0), stop=(ki == 2),
                    )
                dst = qk_sb[ti * 2 + m][:, half * HALF + c0:half * HALF + c0 + cw]
                if c0 == 0:
                    nc.scalar.copy(dst, ps[:, 0:cw])
                else:
                    nc.vector.tensor_copy(dst, ps[:, 0:cw])

        def v_proj(j):
            """v for n-tile j (all 4 heads) -> v1s[j] bf16 with ones cols."""
            ps = smps.tile([128, IPC], F32, tag="sm", name="smv")
            for ki, (k0, kw) in enumerate(KS):
                nc.tensor.matmul(
                    ps[:],
                    lhsT=xts[ki][:kw, j * 128:(j + 1) * 128],
                    rhs=wvs[ki][:kw, :],
                    start=(ki == 0), stop=(ki == 2),
                )
            v1v = v1s[j][:].rearrange("p (h e) -> p h e", e=65)
            nc.gpsimd.memset(v1v[:, :, 64:65], 1.0)
            # alternate ACT/DVE so neither engine eats all 20 copies while
            # also chewing the first block's exp stream
            src = ps[:].rearrange("p (h d) -> p h d", d=64)
            if j % 2 == 0:
                nc.vector.tensor_copy(v1v[:, :, 0:64], src)
            else:
                nc.scalar.copy(v1v[:, :, 0:64], src)

        def scores_part1(h, half, j):
            """s1 half of the scores (cols 0:ACOLS) + the ACT exp -> eta.

            s1 is read only by ACT, s2 only by DVE, so each engine gates only
            its own psum slot chain. ACT is the steady-state pacer, so part1
            is emitted FIRST in each j-body to keep its exps back-to-back.
            """
            m, po = h // 2, (h % 2) * 64
            ps1 = spool.tile([128, ACOLS], F32, tag="s1", name="s1")
            for c0, cw in ((0, 512), (512, 256)):
                nc.tensor.matmul(
                    ps1[:, c0:c0 + cw],
                    lhsT=qk_sb[2 + m][po:po + 64, j * 128:(j + 1) * 128],
                    rhs=qk_sb[m][po:po + 64, half * HALF + c0:half * HALF + c0 + cw],
                    start=True, stop=True,
                )
            eta = epool.tile([128, ACOLS], BF16, tag="ea", name="eta")
            nc.scalar.activation(eta[:], ps1[:], EXP, scale=float(SCALE))
            return eta

        def scores_part2(h, half, j):
            """s2 half of the scores (cols ACOLS:) + the DVE exp -> etb."""
            m, po = h // 2, (h % 2) * 64
            ps2 = spool.tile([128, HALF - ACOLS], F32, tag="s2", name="s2")
            nc.tensor.matmul(
                ps2[:],
                lhsT=qk_sb[2 + m][po:po + 64, j * 128:(j + 1) * 128],
                rhs=qk_sb[m][po:po + 64, half * HALF + ACOLS:(half + 1) * HALF],
                start=True, stop=True,
            )
            etb = epool.tile([128, HALF - ACOLS], BF16, tag="eb", name="etb")
            nc.vector.tensor_scalar(
                etb[:].bitcast(I16), ps2[:],
                float(A_MUL), float(B_ADD),
                mybir.AluOpType.mult, mybir.AluOpType.add)
            return etb

        def attn_mm(h, half, ets, i, tail=False):
            """out'[i-tile] = sum_j expT_j[:, i].T @ [v|1]; normalize to 'on'.

            The transpose+flush is deferred (attn_fin) so the in-order PE
            queue never waits on the DVE normalize of the same step.
            """
            pool = tails["pool"] if tail else smps
            pso = pool.tile([128, 65], F32, tag="to" if tail else "sm", name="smo")
            na = ACOLS // 128
            for j in range(NT):
                eta, etb = ets[j]
                lhsT = (eta[:, i * 128:(i + 1) * 128] if i < na
                        else etb[:, (i - na) * 128:(i - na + 1) * 128])
                nc.tensor.matmul(
                    pso[:],
                    lhsT=lhsT,
                    rhs=v1s[j][:, h * 65:(h + 1) * 65],
                    start=(j == 0), stop=(j == NT - 1),
                )
            rc = sbsm.tile([128, 1], F32, tag="rc", name="rc")
            nc.vector.reciprocal(rc[:], pso[:, 64:65])
            on = sbsm.tile([128, 64], BF16, tag="on", name="on")
            nc.vector.tensor_scalar_mul(on[:], pso[:, 0:64], rc[:])
            return (h, half, on, i)

        def attn_fin(f, tail=False):
            """PE transpose of 'on' + pair-batched outT flush."""
            h, half, on, i = f
            m, po = h // 2, (h % 2) * 64
            pool = tails["pool"] if tail else smps
            if tail:
                # tail is latency-bound: flush each transpose immediately,
                # on ACT (DVE carries the tail norm + y-copy stream)
                pst = pool.tile([128, 128], BF16, tag="tt", name="smt",
                                bufs=1)
                nc.tensor.transpose(pst[0:64, 0:128], on[:], identb[:])
                ig = half * 10 + i
                nc.vector.tensor_copy(
                    outT[m][po:po + 64, ig * 128:(ig + 1) * 128],
                    pst[0:64, 0:128])
                return
            # pair-batched transposes: even i allocates a [64,256] bf16 psum
            # tile, odd i completes it and flushes both to outT in one copy.
            if i % 2 == 0:
                tstate["grp"] = pool.tile([128, 256], BF16, tag="sm", name="smt")
            pst = tstate["grp"]
            nc.tensor.transpose(pst[0:64, (i % 2) * 128:(i % 2) * 128 + 128],
                                on[:], identb[:])
            if i % 2 == 1:
                ig0 = half * 10 + i - 1
                nc.vector.tensor_copy(
                    outT[m][po:po + 64, ig0 * 128:(ig0 + 2) * 128],
                    pst[0:64, 0:256])

        def y_step(i, tail=False):
            """y[i-tile] = outT[:, i].T @ Wo -> DRAM."""
            psy = (tails["pool"].tile([128, QD], F32, tag="ty", name="smy",
                                       bufs=3)
                   if tail else smps.tile([128, QD], F32, tag="sm", name="smy"))
            for kk in range(2):
                nc.tensor.matmul(
                    psy[:],
                    lhsT=outT[kk][:, i * 128:(i + 1) * 128],
                    rhs=wos[kk][:],
                    start=(kk == 0), stop=(kk == 1),
                )
            ysb = ypool.tile([128, QD], F32, tag="y", name="ysb")
            nc.scalar.copy(ysb[:], psy[:])
            nc.sync.dma_start(y[i * 128:i * 128 + 64, :], ysb[0:64, :])
            nc.gpsimd.dma_start(y[i * 128 + 64:(i + 1) * 128, :], ysb[64:128, :])

        # ---- emission: minimal upfront proj, rest interleaved ----
        # k cols 0:128 + q chunk A unblock the first scores matmul ASAP
        qk_proj(1, 0, 0, chunks=[(0, 128)])
        qk_proj(0, 0, 0)
        qk_proj(1, 0, 0, chunks=[(128, 384), (512, 512), (1024, 256)])
        # remaining projection slabs, one CHUNK per slot; slots chosen on
        # even j that do NOT carry a pair-tile allocation (j % 4 == 2 does)
        # so the smps "sm" slot chain never stalls the in-order PE queue
        pend_list = [
            ((0, 0, 0), (1, 0, 1), 0), ((0, 0, 2), (1, 0, 1), 1),
            ((0, 0, 4), (1, 0, 1), 2), ((0, 0, 6), (0, 0, 1), 0),
            ((0, 0, 8), (0, 0, 1), 1), ((0, 0, 10), (0, 0, 1), 2),
            ((0, 1, 4), (0, 1, 0), 0), ((0, 1, 8), (0, 1, 0), 1),
            ((0, 1, 12), (0, 1, 0), 2), ((0, 1, 16), (1, 1, 0), 0),
            ((1, 0, 0), (1, 1, 0), 1), ((1, 0, 4), (1, 1, 0), 2),
            ((1, 0, 8), (1, 1, 1), 0), ((1, 0, 12), (1, 1, 1), 1),
            ((1, 0, 16), (1, 1, 1), 2), ((1, 1, 0), (0, 1, 1), 0),
            ((1, 1, 4), (0, 1, 1), 1), ((1, 1, 8), (0, 1, 1), 2),
        ]
        pending = {slot: (slab, ci) for slot, slab, ci in pend_list}

        fin_q = []
        prev = None
        for h in range(HPC):
            for half in range(2):
                ets = []
                for j in range(NT):
                    # ready PE work first so the spool-gated scores matmuls
                    # sit last in the in-order PE queue
                    if j % 2 == 0 and fin_q:
                        # (y-steps are all deferred to the tail so the (3,1)
                        # block's ACT stream stays exp-only.)
                        attn_fin(fin_q.pop(0))
                    pr = pending.pop((h, half, j), None)
                    if pr is not None:
                        (ti, m, ph_), ci = pr
                        qk_proj(ti, m, ph_, chunks=CHUNKS[ci:ci + 1])
                    if h == 0 and half == 0:
                        v_proj(j)
                    elif prev is not None and j % 2 == 1:
                        ph, phalf, pets = prev
                        fin_q.append(attn_mm(ph, phalf, pets, j // 2))
                    ets.append((scores_part1(h, half, j),
                                scores_part2(h, half, j)))
                prev = (h, half, ets)
        spool_cm.__exit__(None, None, None)
        tpool = ctx.enter_context(tc.tile_pool(name="tpool", bufs=2, space="PSUM"))
        tails["pool"] = tpool

        def drain_fin(tail):
            f = fin_q.pop(0)
            attn_fin(f, tail=tail)
            if f[0] == 3 and f[1] == 1:
                y_step(10 + f[3], tail=True)

        # leftover fin from the main loop ((3,0) attn i=9)
        if fin_q:
            drain_fin(False)
        # tail: (3,1) attn pipelined with BOTH y half-streams. i>=6 attn
        # reads only the DVE-produced etb tiles (ready before ACT's last
        # exps), and the deferred first-half y's are ready immediately, so
        # both fill the queue while ACT drains its exp backlog.
        yq = list(range(10))
        for idx, i in enumerate([6, 7, 8, 9, 0, 1, 2, 3, 4, 5]):
            fin_q.append(attn_mm(3, 1, prev[2], i, tail=True))
            for _ in range(2):
                if yq:
                    y_step(yq.pop(0), tail=True)
            if idx >= 1:
                drain_fin(True)
        while fin_q:
            drain_fin(True)

    nc.compile()
    return nc


def _get_nc():
    if "nc" not in _built:
        _built["nc"] = _build()
    return _built["nc"]


def kernel(x, Wq, Wk, Wv, Wo, bo):
    global last_results
    import ml_dtypes
    x = np.asarray(x, dtype=np.float32)
    Wq = np.asarray(Wq, dtype=np.float32)
    Wk = np.asarray(Wk, dtype=np.float32)
    Wv = np.asarray(Wv, dtype=np.float32)
    Wo = np.asarray(Wo, dtype=np.float32)
    bo = np.asarray(bo, dtype=np.float32)

    nc = _get_nc()
    in_maps = []
    for c in range(8):
        bb, g = divmod(c, 2)
        sl = slice(g * IPC, (g + 1) * IPC)
        in_maps.append({
            "xT": np.ascontiguousarray(x[bb].T).astype(ml_dtypes.bfloat16),
            "wq": np.ascontiguousarray(Wq[:, sl]).astype(ml_dtypes.bfloat16),
            "wk": np.ascontiguousarray(Wk[:, sl]).astype(ml_dtypes.bfloat16),
            "wv": np.ascontiguousarray(Wv[:, sl]).astype(ml_dtypes.bfloat16),
            "wo": np.ascontiguousarray(Wo[sl, :]).astype(ml_dtypes.bfloat16),
        })
    res = run_bass_kernel_spmd(nc, in_maps, core_ids=list(range(8)))
    last_results = res
    parts = [r["y"] for r in res.results]
    out = np.empty((B, N, QD), dtype=np.float32)
    for bb in range(B):
        out[bb] = parts[2 * bb] + parts[2 * bb + 1]
    out += bo
    return out


# revision 4
# speedup vs baseline: 1.0261x; 1.0261x over previous
"""Fused multi-head self-attention kernel for Trainium2 (Bass/Tile). v2.

Problem: x:[4,2560,320] f32, Wq/Wk/Wv:[320,512], Wo:[512,320], bo:[320]
  q,k,v = x@W*, 8 heads x 64; sim = q k^T * d^-0.5; attn = softmax(sim);
  out = (attn @ v) @ Wo + bo.

Sharding: batch*head 32-way -> 8 cores: core c handles batch c//2 and the
4-head group c%2. Host sums the two half-head partial output projections
per batch and adds the bias.

Per-core layout trick: scores are computed TRANSPOSED (sT[j,i] = k_j . q_i)
so that the softmax denominator arrives for free: v gets a ones-column
appended, and out' = expT_slice.T @ [v|1] accumulates both attn@v and the
row sums. Normalisation is a per-partition reciprocal+multiply.

Engine budget (cost model, per core): PE ~160us is the floor (scores
204.8k cyc + attn@v 104k + projections/transposes ~69k). The ACT-engine
exp over 26M elements (~218us alone -- the old bottleneck) is split:
 - ACT computes exp on score cols [0:ACOLS] natively; DVE computes cols
   [ACOLS:1280] with a one-instruction Schraudolph bitcast exp
   (int16(x*A+B) viewed as bf16; ~1.8% rms there, 40% of weights).
 - eta/etb live in SEPARATE tiles and the scores land in two psum tiles
   (s1 ACT-read / s2 DVE-read), so neither engine's slot chain gates the
   other (shared tiles serialize via write-write/read hazards).
 - x/Wq/Wk/Wv/Wo are bf16 (host-side cast): halves DMA + SBUF footprint.
 - attn normalize ('on') is bf16 so PE transposes run at 1 cyc/row; outT
   is bf16; transposes flushed in pairs; attn transpose+flush deferred one
   step so the in-order PE queue never waits on the DVE normalize.
 - all y-steps run in the tail (keeps the last block's ACT stream
   exp-only); tail attn runs i>=6 first (needs only DVE-produced etb).
"""

import sys

import numpy as np

if "/opt/trn_rl_repo" not in sys.path:
    sys.path.insert(0, "/opt/trn_rl_repo")

from contextlib import ExitStack

import concourse.bass as bass
from concourse import bacc
import concourse.mybir as mybir
import concourse.tile as tile
from concourse.bass_utils import run_bass_kernel_spmd
from concourse.masks import make_identity

# ---- problem constants (hardcoded per contract) ----
B = 4
N = 2560
QD = 320
H_TOT = 8
D = 64
HPC = 4                  # heads per core
IPC = HPC * D            # 256 inner dims per core
SCALE = D ** -0.5
NT = N // 128            # 20 n-tiles
HALF = N // 2            # 1280
F32 = mybir.dt.float32
BF16 = mybir.dt.bfloat16
F32R = mybir.dt.float32r
I16 = mybir.dt.int16
EXP = mybir.ActivationFunctionType.Exp

# qd (=320) split into K subtiles for the 128-partition contraction
KS = [(0, 128), (128, 128), (256, 64)]
# 1280-wide column chunks (PSUM-bank-aligned matmul N<=512)
CHUNKS = [(0, 512), (512, 512), (1024, 256)]

# exp split: ACT native exp on [0:ACOLS], DVE Schraudolph on [ACOLS:HALF].
# ACOLS must be a multiple of 128 so the two exp outputs can live in
# SEPARATE tiles (shared-tile writes serialize ACT behind DVE via a
# write-write hazard) with attnv i-tiles reading wholly from one of them.
ACOLS = 768
A_MUL = 128.0 * SCALE / np.log(2.0)       # folds the 1/sqrt(d) scale
B_ADD = 128.0 * (127.0 - 0.058)           # rounding-convert-optimal bias

EXP_BUFS = 43            # expT pool slots of [128,1280] bf16 (2.5KB/part each)

_built = {}
last_results = None      # stashed BassKernelResults for the test harness


def _build():
    nc = bacc.Bacc(None, target_bir_lowering=False)
    xT = nc.declare_dram_parameter("xT", [QD, N], BF16, isOutput=False)
    wq = nc.declare_dram_parameter("wq", [QD, IPC], BF16, isOutput=False)
    wk = nc.declare_dram_parameter("wk", [QD, IPC], BF16, isOutput=False)
    wv = nc.declare_dram_parameter("wv", [QD, IPC], BF16, isOutput=False)
    wo = nc.declare_dram_parameter("wo", [IPC, QD], BF16, isOutput=False)
    y = nc.declare_dram_parameter("y", [N, QD], F32, isOutput=True)

    with tile.TileContext(nc) as tc, ExitStack() as ctx:
        const = ctx.enter_context(tc.tile_pool(name="const", bufs=1))
        smps = ctx.enter_context(tc.tile_pool(name="smps", bufs=2, space="PSUM"))
        epool = ctx.enter_context(tc.tile_pool(name="epool", bufs=EXP_BUFS))
        sbsm = ctx.enter_context(tc.tile_pool(name="sbsm", bufs=4))
        ypool = ctx.enter_context(tc.tile_pool(name="ypool", bufs=5))
        spool_cm = tc.tile_pool(name="spool", bufs=2, space="PSUM")
        spool = spool_cm.__enter__()

        ident = const.tile([128, 128], F32, tag="ident", name="ident")
        make_identity(nc, ident[:])
        identb = const.tile([128, 128], BF16, tag="identb", name="identb")
        nc.vector.tensor_copy(identb[:], ident[:])
        warm = sbsm.tile([128, 1], F32, tag="rc", name="warm")
        nc.scalar.activation(warm[:], ident[:, 0:1], EXP, scale=1.0)
        for _ in range(6):
            pw = smps.tile([128, 128], F32, tag="sm", name="pwarm")
            nc.tensor.matmul(pw[:], lhsT=ident[:], rhs=ident[:],
                             start=True, stop=True)

        # ---- persistent inputs (DMA emission ordered by first use) ----
        xts = [const.tile([128, N], BF16, tag=f"xt{ki}", name=f"xt{ki}")
               for ki in range(3)]
        wqs = [const.tile([128, IPC], BF16, tag=f"wq{ki}", name=f"wq{ki}")
               for ki in range(3)]
        wks = [const.tile([128, IPC], BF16, tag=f"wk{ki}", name=f"wk{ki}")
               for ki in range(3)]
        wvs = [const.tile([128, IPC], BF16, tag=f"wv{ki}", name=f"wv{ki}")
               for ki in range(3)]
        wos = [const.tile([128, QD], BF16, tag=f"wo{kk}", name=f"wo{kk}")
               for kk in range(2)]
        # critical set first; x split across BOTH dma queues (sync + gpsimd)
        for ki, (k0, kw) in enumerate(KS):
            nc.sync.dma_start(xts[ki][:kw, 0:640], xT[k0:k0 + kw, 0:640])
            nc.gpsimd.dma_start(wqs[ki][:kw, :], wq[k0:k0 + kw, :])
        for ki, (k0, kw) in enumerate(KS):
            nc.sync.dma_start(xts[ki][:kw, 640:1280], xT[k0:k0 + kw, 640:1280])
            nc.gpsimd.dma_start(wks[ki][:kw, :], wk[k0:k0 + kw, :])
        for ki, (k0, kw) in enumerate(KS):
            nc.gpsimd.dma_start(xts[ki][:kw, 1280:1920], xT[k0:k0 + kw, 1280:1920])
            nc.sync.dma_start(xts[ki][:kw, 1920:2560], xT[k0:k0 + kw, 1920:2560])
        for ki, (k0, kw) in enumerate(KS):
            nc.gpsimd.dma_start(wvs[ki][:kw, :], wv[k0:k0 + kw, :])
        for kk in range(2):
            nc.gpsimd.dma_start(wos[kk][:], wo[kk * 128:(kk + 1) * 128, :])

        # qT/kT: [inner(256) x n] as 2 tiles of [128, N] each; fp32 storage
        qk_sb = [const.tile([128, N], F32R, tag=f"qk{i}", name=f"qk{i}") for i in range(4)]
        # outT: normalized attention output, [inner x n], bf16
        outT = [const.tile([128, N], BF16, tag=f"oT{kk}", name=f"oT{kk}") for kk in range(2)]
        # v with ones column per head: [n-tile][128, 4*65] bf16
        v1s = [const.tile([128, HPC * 65], BF16, tag=f"v1_{j}", name=f"v1_{j}") for j in range(NT)]

        ws = [wqs, wks]
        tails = {}
        tstate = {}

        def qk_proj(ti, m, half, chunks=None):
            """qT/kT tile ti(0=q,1=k), inner slab m, col half -> qk_sb[ti*2+m].

            PSUM->SBUF copies: 512-wide chunks go to ACT (has startup slack),
            the 256-wide chunk to DVE, to keep early-block DVE load down.
            """
            for c0, cw in (chunks or CHUNKS):
                ps = smps.tile([128, 512], F32, tag="sm", name="smp")
                for ki, (k0, kw) in enumerate(KS):
                    nc.tensor.matmul(
                        ps[:, 0:cw],
                        lhsT=ws[ti][ki][:kw, m * 128:(m + 1) * 128],
                        rhs=xts[ki][:kw, half * HALF + c0:half * HALF + c0 + cw],
                        start=(ki == 0), stop=(ki == 2),
                    )
                dst = qk_sb[ti * 2 + m][:, half * HALF + c0:half * HALF + c0 + cw]
                if c0 == 0:
                    nc.scalar.copy(dst, ps[:, 0:cw])
                else:
                    nc.vector.tensor_copy(dst, ps[:, 0:cw])

        def v_proj(j):
            """v for n-tile j (all 4 heads) -> v1s[j] bf16 with ones cols."""
            ps = smps.tile([128, IPC], F32, tag="sm", name="smv")
            for ki, (k0, kw) in enumerate(KS):
                nc.tensor.matmul(
                    ps[:],
                    lhsT=xts[ki][:kw, j * 128:(j + 1) * 128],
                    rhs=wvs[ki][:kw, :],
                    start=(ki == 0), stop=(ki == 2),
                )
            v1v = v1s[j][:].rearrange("p (h e) -> p h e", e=65)
            nc.gpsimd.memset(v1v[:, :, 64:65], 1.0)
            # alternate ACT/DVE so neither engine eats all 20 copies while
            # also chewing the first block's exp stream
            src = ps[:].rearrange("p (h d) -> p h d", d=64)
            if j % 2 == 0:
                nc.vector.tensor_copy(v1v[:, :, 0:64], src)
            else:
                nc.scalar.copy(v1v[:, :, 0:64], src)

        def scores_part1(h, half, j):
            """s1 half of the scores (cols 0:ACOLS) + the ACT exp -> eta.

            s1 is read only by ACT, s2 only by DVE, so each engine gates only
            its own psum slot chain. ACT is the steady-state pacer, so part1
            is emitted FIRST in each j-body to keep its exps back-to-back.
            """
            m, po = h // 2, (h % 2) * 64
            ps1 = spool.tile([128, ACOLS], F32, tag="s1", name="s1")
            for c0, cw in ((0, 512), (512, 256)):
                nc.tensor.matmul(
                    ps1[:, c0:c0 + cw],
                    lhsT=qk_sb[2 + m][po:po + 64, j * 128:(j + 1) * 128],
                    rhs=qk_sb[m][po:po + 64, half * HALF + c0:half * HALF + c0 + cw],
                    start=True, stop=True,
                )
            eta = epool.tile([128, ACOLS], BF16, tag="ea", name="eta")
            nc.scalar.activation(eta[:], ps1[:], EXP, scale=float(SCALE))
            return eta

        def scores_part2(h, half, j):
            """s2 half of the scores (cols ACOLS:) + the DVE exp -> etb."""
            m, po = h // 2, (h % 2) * 64
            ps2 = spool.tile([128, HALF - ACOLS], F32, tag="s2", name="s2")
            nc.tensor.matmul(
                ps2[:],
                lhsT=qk_sb[2 + m][po:po + 64, j * 128:(j + 1) * 128],
                rhs=qk_sb[m][po:po + 64, half * HALF + ACOLS:(half + 1) * HALF],
                start=True, stop=True,
            )
            etb = epool.tile([128, HALF - ACOLS], BF16, tag="eb", name="etb")
            nc.vector.tensor_scalar(
                etb[:].bitcast(I16), ps2[:],
                float(A_MUL), float(B_ADD),
                mybir.AluOpType.mult, mybir.AluOpType.add)
            return etb

        def attn_mm(h, half, ets, i, tail=False):
            """out'[i-tile] = sum_j expT_j[:, i].T @ [v|1]; normalize to 'on'.

            The transpose+flush is deferred (attn_fin) so the in-order PE
            queue never waits on the DVE normalize of the same step.
            """
            pool = tails["pool"] if tail else smps
            pso = pool.tile([128, 65], F32, tag="to" if tail else "sm", name="smo")
            na = ACOLS // 128
            for j in range(NT):
                eta, etb = ets[j]
                lhsT = (eta[:, i * 128:(i + 1) * 128] if i < na
                        else etb[:, (i - na) * 128:(i - na + 1) * 128])
                nc.tensor.matmul(
                    pso[:],
                    lhsT=lhsT,
                    rhs=v1s[j][:, h * 65:(h + 1) * 65],
                    start=(j == 0), stop=(j == NT - 1),
                )
            rc = sbsm.tile([128, 1], F32, tag="rc", name="rc")
            nc.vector.reciprocal(rc[:], pso[:, 64:65])
            on = sbsm.tile([128, 64], BF16, tag="on", name="on")
            nc.vector.tensor_scalar_mul(on[:], pso[:, 0:64], rc[:])
            return (h, half, on, i)

        def attn_fin(f, tail=False):
            """PE transpose of 'on' + pair-batched outT flush."""
            h, half, on, i = f
            m, po = h // 2, (h % 2) * 64
            pool = tails["pool"] if tail else smps
            if tail:
                # tail is latency-bound: flush each transpose immediately,
                # on ACT (DVE carries the tail norm + y-copy stream)
                pst = pool.tile([128, 128], BF16, tag="tt", name="smt",
                                bufs=1)
                nc.tensor.transpose(pst[0:64, 0:128], on[:], identb[:])
                ig = half * 10 + i
                nc.vector.tensor_copy(
                    outT[m][po:po + 64, ig * 128:(ig + 1) * 128],
                    pst[0:64, 0:128])
                return
            # pair-batched transposes: even i allocates a [64,256] bf16 psum
            # tile, odd i completes it and flushes both to outT in one copy.
            if i % 2 == 0:
                tstate["grp"] = pool.tile([128, 256], BF16, tag="sm", name="smt")
            pst = tstate["grp"]
            nc.tensor.transpose(pst[0:64, (i % 2) * 128:(i % 2) * 128 + 128],
                                on[:], identb[:])
            if i % 2 == 1:
                ig0 = half * 10 + i - 1
                nc.vector.tensor_copy(
                    outT[m][po:po + 64, ig0 * 128:(ig0 + 2) * 128],
                    pst[0:64, 0:256])

        def y_step(i, tail=False):
            """y[i-tile] = outT[:, i].T @ Wo -> DRAM."""
            psy = (tails["pool"].tile([128, QD], F32, tag="ty", name="smy",
                                       bufs=3)
                   if tail else smps.tile([128, QD], F32, tag="sm", name="smy"))
            for kk in range(2):
                nc.tensor.matmul(
                    psy[:],
                    lhsT=outT[kk][:, i * 128:(i + 1) * 128],
                    rhs=wos[kk][:],
                    start=(kk == 0), stop=(kk == 1),
                )
            ysb = ypool.tile([128, QD], F32, tag="y", name="ysb")
            nc.scalar.copy(ysb[:], psy[:])
            nc.sync.dma_start(y[i * 128:i * 128 + 64, :], ysb[0:64, :])
            nc.gpsimd.dma_start(y[i * 128 + 64:(i + 1) * 128, :], ysb[64:128, :])

        # ---- emission: minimal upfront proj, rest interleaved ----
        # k cols 0:128 + q chunk A unblock the first scores matmul ASAP
        qk_proj(1, 0, 0, chunks=[(0, 128)])
        qk_proj(0, 0, 0)
        qk_proj(1, 0, 0, chunks=[(128, 384)])
        # remaining projection slabs, one CHUNK per slot; slots chosen on
        # even j that do NOT carry a pair-tile allocation (j % 4 == 2 does)
        # so the smps "sm" slot chain never stalls the in-order PE queue
        pend_list = [
            ((0, 0, 0), (1, 0, 0), 4), ((0, 0, 2), (1, 0, 0), 5),
            ((0, 0, 4), (1, 0, 1), 0), ((0, 0, 6), (1, 0, 1), 1),
            ((0, 0, 8), (1, 0, 1), 2), ((0, 0, 10), (0, 0, 1), 0),
            ((0, 0, 12), (0, 0, 1), 1), ((0, 0, 14), (0, 0, 1), 2),
            ((0, 1, 4), (0, 1, 0), 0), ((0, 1, 8), (0, 1, 0), 1),
            ((0, 1, 12), (0, 1, 0), 2), ((0, 1, 16), (1, 1, 0), 0),
            ((1, 0, 0), (1, 1, 0), 1), ((1, 0, 4), (1, 1, 0), 2),
            ((1, 0, 8), (1, 1, 1), 0), ((1, 0, 12), (1, 1, 1), 1),
            ((1, 0, 16), (1, 1, 1), 2), ((1, 1, 0), (0, 1, 1), 0),
            ((1, 1, 4), (0, 1, 1), 1), ((1, 1, 8), (0, 1, 1), 2),
        ]
        pending = {slot: (slab, ci) for slot, slab, ci in pend_list}

        fin_q = []
        prev = None
        for h in range(HPC):
            for half in range(2):
                ets = []
                for j in range(NT):
                    # ready PE work first so the spool-gated scores matmuls
                    # sit last in the in-order PE queue
                    if j % 2 == 0 and fin_q:
                        # (y-steps are all deferred to the tail so the (3,1)
                        # block's ACT stream stays exp-only.)
                        attn_fin(fin_q.pop(0))
                    pr = pending.pop((h, half, j), None)
                    if pr is not None:
                        (ti, m, ph_), ci = pr
                        allc = CHUNKS + [(0, 128), (512, 512), (1024, 256)]
                        qk_proj(ti, m, ph_, chunks=allc[ci:ci + 1])
                    if h == 0 and half == 0:
                        v_proj(j)
                    elif prev is not None and j % 2 == 1:
                        ph, phalf, pets = prev
                        fin_q.append(attn_mm(ph, phalf, pets, j // 2))
                    ets.append((scores_part1(h, half, j),
                                scores_part2(h, half, j)))
                prev = (h, half, ets)
        spool_cm.__exit__(None, None, None)
        tpool = ctx.enter_context(tc.tile_pool(name="tpool", bufs=2, space="PSUM"))
        tails["pool"] = tpool

        def drain_fin(tail):
            f = fin_q.pop(0)
            attn_fin(f, tail=tail)
            if f[0] == 3 and f[1] == 1:
                y_step(10 + f[3], tail=True)

        # leftover fin from the main loop ((3,0) attn i=9)
        if fin_q:
            drain_fin(False)
        # tail: (3,1) attn pipelined with BOTH y half-streams. i>=6 attn
        # reads only the DVE-produced etb tiles (ready before ACT's last
        # exps), and the deferred first-half y's are ready immediately, so
        # both fill the queue while ACT drains its exp backlog.
        yq = list(range(10))
        for idx, i in enumerate([6, 7, 8, 9, 0, 1, 2, 3, 4, 5]):
            fin_q.append(attn_mm(3, 1, prev[2], i, tail=True))
            if yq:
                y_step(yq.pop(0), tail=True)
            if idx >= 1:
                drain_fin(True)
        while fin_q:
            drain_fin(True)

    nc.compile()
    return nc


def _get_nc():
    if "nc" not in _built:
        _built["nc"] = _build()
    return _built["nc"]


def kernel(x, Wq, Wk, Wv, Wo, bo):
    global last_results
    import ml_dtypes
    x = np.asarray(x, dtype=np.float32)
    Wq = np.asarray(Wq, dtype=np.float32)
    Wk = np.asarray(Wk, dtype=np.float32)
    Wv = np.asarray(Wv, dtype=np.float32)
    Wo = np.asarray(Wo, dtype=np.float32)
    bo = np.asarray(bo, dtype=np.float32)

    nc = _get_nc()
    in_maps = []
    for c in range(8):
        bb, g = divmod(c, 2)
        sl = slice(g * IPC, (g + 1) * IPC)
        in_maps.append({
            "xT": np.ascontiguousarray(x[bb].T).astype(ml_dtypes.bfloat16),
            "wq": np.ascontiguousarray(Wq[:, sl]).astype(ml_dtypes.bfloat16),
            "wk": np.ascontiguousarray(Wk[:, sl]).astype(ml_dtypes.bfloat16),
            "wv": np.ascontiguousarray(Wv[:, sl]).astype(ml_dtypes.bfloat16),
            "wo": np.ascontiguousarray(Wo[sl, :]).astype(ml_dtypes.bfloat16),
        })
    res = run_bass_kernel_spmd(nc, in_maps, core_ids=list(range(8)))
    last_results = res
    parts = [r["y"] for r in res.results]
    out = np.empty((B, N, QD), dtype=np.float32)
    for bb in range(B):
        out[bb] = parts[2 * bb] + parts[2 * bb + 1]
    out += bo
    return out


# revision 5
# speedup vs baseline: 1.0361x; 1.0097x over previous
"""Fused multi-head self-attention kernel for Trainium2 (Bass/Tile). v2.

Problem: x:[4,2560,320] f32, Wq/Wk/Wv:[320,512], Wo:[512,320], bo:[320]
  q,k,v = x@W*, 8 heads x 64; sim = q k^T * d^-0.5; attn = softmax(sim);
  out = (attn @ v) @ Wo + bo.

Sharding: batch*head 32-way -> 8 cores: core c handles batch c//2 and the
4-head group c%2. Host sums the two half-head partial output projections
per batch and adds the bias.

Per-core layout trick: scores are computed TRANSPOSED (sT[j,i] = k_j . q_i)
so that the softmax denominator arrives for free: v gets a ones-column
appended, and out' = expT_slice.T @ [v|1] accumulates both attn@v and the
row sums. Normalisation is a per-partition reciprocal+multiply.

Engine budget (cost model, per core): PE ~160us is the floor (scores
204.8k cyc + attn@v 104k + projections/transposes ~69k). The ACT-engine
exp over 26M elements (~218us alone -- the old bottleneck) is split:
 - ACT computes exp on score cols [0:ACOLS] natively; DVE computes cols
   [ACOLS:1280] with a one-instruction Schraudolph bitcast exp
   (int16(x*A+B) viewed as bf16; ~1.8% rms there, 40% of weights).
 - eta/etb live in SEPARATE tiles and the scores land in two psum tiles
   (s1 ACT-read / s2 DVE-read), so neither engine's slot chain gates the
   other (shared tiles serialize via write-write/read hazards).
 - x/Wq/Wk/Wv/Wo are bf16 (host-side cast): halves DMA + SBUF footprint.
 - attn normalize ('on') is bf16 so PE transposes run at 1 cyc/row; outT
   is bf16; transposes flushed in pairs; attn transpose+flush deferred one
   step so the in-order PE queue never waits on the DVE normalize.
 - all y-steps run in the tail (keeps the last block's ACT stream
   exp-only); tail attn runs i>=6 first (needs only DVE-produced etb).
"""

import sys

import numpy as np

if "/opt/trn_rl_repo" not in sys.path:
    sys.path.insert(0, "/opt/trn_rl_repo")

from contextlib import ExitStack

import concourse.bass as bass
from concourse import bacc
import concourse.mybir as mybir
import concourse.tile as tile
from concourse.bass_utils import run_bass_kernel_spmd
from concourse.masks import make_identity

# ---- problem constants (hardcoded per contract) ----
B = 4
N = 2560
QD = 320
H_TOT = 8
D = 64
HPC = 4                  # heads per core
IPC = HPC * D            # 256 inner dims per core
SCALE = D ** -0.5
NT = N // 128            # 20 n-tiles
HALF = N // 2            # 1280
F32 = mybir.dt.float32
BF16 = mybir.dt.bfloat16
F32R = mybir.dt.float32r
I16 = mybir.dt.int16
EXP = mybir.ActivationFunctionType.Exp

# qd (=320) split into K subtiles for the 128-partition contraction
KS = [(0, 128), (128, 128), (256, 64)]
# 1280-wide column chunks (PSUM-bank-aligned matmul N<=512)
CHUNKS = [(0, 512), (512, 512), (1024, 256)]

# exp split: ACT native exp on [0:ACOLS], DVE Schraudolph on [ACOLS:HALF].
# ACOLS must be a multiple of 128 so the two exp outputs can live in
# SEPARATE tiles (shared-tile writes serialize ACT behind DVE via a
# write-write hazard) with attnv i-tiles reading wholly from one of them.
ACOLS = 768
A_MUL = 128.0 * SCALE / np.log(2.0)       # folds the 1/sqrt(d) scale
B_ADD = 128.0 * (127.0 - 0.058)           # rounding-convert-optimal bias

EXP_BUFS = 43            # expT pool slots of [128,1280] bf16 (2.5KB/part each)

_built = {}
last_results = None      # stashed BassKernelResults for the test harness


def _build():
    nc = bacc.Bacc(None, target_bir_lowering=False)
    xT = nc.declare_dram_parameter("xT", [QD, N], BF16, isOutput=False)
    wq = nc.declare_dram_parameter("wq", [QD, IPC], BF16, isOutput=False)
    wk = nc.declare_dram_parameter("wk", [QD, IPC], BF16, isOutput=False)
    wv = nc.declare_dram_parameter("wv", [QD, IPC], BF16, isOutput=False)
    wo = nc.declare_dram_parameter("wo", [IPC, QD], BF16, isOutput=False)
    y = nc.declare_dram_parameter("y", [N, QD], F32, isOutput=True)

    with tile.TileContext(nc) as tc, ExitStack() as ctx:
        const = ctx.enter_context(tc.tile_pool(name="const", bufs=1))
        smps = ctx.enter_context(tc.tile_pool(name="smps", bufs=2, space="PSUM"))
        epool = ctx.enter_context(tc.tile_pool(name="epool", bufs=EXP_BUFS))
        sbsm = ctx.enter_context(tc.tile_pool(name="sbsm", bufs=4))
        ypool = ctx.enter_context(tc.tile_pool(name="ypool", bufs=5))
        spool_cm = tc.tile_pool(name="spool", bufs=2, space="PSUM")
        spool = spool_cm.__enter__()

        ident = const.tile([128, 128], F32, tag="ident", name="ident")
        make_identity(nc, ident[:])
        identb = const.tile([128, 128], BF16, tag="identb", name="identb")
        nc.vector.tensor_copy(identb[:], ident[:])
        warm = sbsm.tile([128, 1], F32, tag="rc", name="warm")
        nc.scalar.activation(warm[:], ident[:, 0:1], EXP, scale=1.0)
        for _ in range(6):
            pw = smps.tile([128, 128], F32, tag="sm", name="pwarm")
            nc.tensor.matmul(pw[:], lhsT=ident[:], rhs=ident[:],
                             start=True, stop=True)

        # ---- persistent inputs (DMA emission ordered by first use) ----
        xts = [const.tile([128, N], BF16, tag=f"xt{ki}", name=f"xt{ki}")
               for ki in range(3)]
        wqs = [const.tile([128, IPC], BF16, tag=f"wq{ki}", name=f"wq{ki}")
               for ki in range(3)]
        wks = [const.tile([128, IPC], BF16, tag=f"wk{ki}", name=f"wk{ki}")
               for ki in range(3)]
        wvs = [const.tile([128, IPC], BF16, tag=f"wv{ki}", name=f"wv{ki}")
               for ki in range(3)]
        wos = [const.tile([128, QD], BF16, tag=f"wo{kk}", name=f"wo{kk}")
               for kk in range(2)]
        # critical set first; x split across BOTH dma queues (sync + gpsimd)
        for ki, (k0, kw) in enumerate(KS):
            nc.sync.dma_start(xts[ki][:kw, 0:640], xT[k0:k0 + kw, 0:640])
            nc.gpsimd.dma_start(wqs[ki][:kw, :], wq[k0:k0 + kw, :])
        for ki, (k0, kw) in enumerate(KS):
            nc.sync.dma_start(xts[ki][:kw, 640:1280], xT[k0:k0 + kw, 640:1280])
            nc.gpsimd.dma_start(wks[ki][:kw, :], wk[k0:k0 + kw, :])
        for ki, (k0, kw) in enumerate(KS):
            nc.gpsimd.dma_start(xts[ki][:kw, 1280:1920], xT[k0:k0 + kw, 1280:1920])
            nc.sync.dma_start(xts[ki][:kw, 1920:2560], xT[k0:k0 + kw, 1920:2560])
        for ki, (k0, kw) in enumerate(KS):
            nc.gpsimd.dma_start(wvs[ki][:kw, :], wv[k0:k0 + kw, :])
        for kk in range(2):
            nc.gpsimd.dma_start(wos[kk][:], wo[kk * 128:(kk + 1) * 128, :])

        # qT/kT: [inner(256) x n] as 2 tiles of [128, N] each; fp32 storage
        qk_sb = [const.tile([128, N], F32R, tag=f"qk{i}", name=f"qk{i}") for i in range(4)]
        # outT: normalized attention output, [inner x n], bf16
        outT = [const.tile([128, N], BF16, tag=f"oT{kk}", name=f"oT{kk}") for kk in range(2)]
        # v with ones column per head: [n-tile][128, 4*65] bf16
        v1s = [const.tile([128, HPC * 65], BF16, tag=f"v1_{j}", name=f"v1_{j}") for j in range(NT)]

        ws = [wqs, wks]
        tails = {}
        tstate = {}

        def qk_proj(ti, m, half, chunks=None):
            """qT/kT tile ti(0=q,1=k), inner slab m, col half -> qk_sb[ti*2+m].

            PSUM->SBUF copies: 512-wide chunks go to ACT (has startup slack),
            the 256-wide chunk to DVE, to keep early-block DVE load down.
            """
            for c0, cw in (chunks or CHUNKS):
                ps = smps.tile([128, 512], F32, tag="sm", name="smp")
                for ki, (k0, kw) in enumerate(KS):
                    nc.tensor.matmul(
                        ps[:, 0:cw],
                        lhsT=ws[ti][ki][:kw, m * 128:(m + 1) * 128],
                        rhs=xts[ki][:kw, half * HALF + c0:half * HALF + c0 + cw],
                        start=(ki == 0), stop=(ki == 2),
                    )
                dst = qk_sb[ti * 2 + m][:, half * HALF + c0:half * HALF + c0 + cw]
                if c0 == 0:
                    nc.scalar.copy(dst, ps[:, 0:cw])
                else:
                    nc.vector.tensor_copy(dst, ps[:, 0:cw])

        def v_proj(j):
            """v for n-tile j (all 4 heads) -> v1s[j] bf16 with ones cols."""
            ps = smps.tile([128, IPC], F32, tag="sm", name="smv")
            for ki, (k0, kw) in enumerate(KS):
                nc.tensor.matmul(
                    ps[:],
                    lhsT=xts[ki][:kw, j * 128:(j + 1) * 128],
                    rhs=wvs[ki][:kw, :],
                    start=(ki == 0), stop=(ki == 2),
                )
            v1v = v1s[j][:].rearrange("p (h e) -> p h e", e=65)
            nc.gpsimd.memset(v1v[:, :, 64:65], 1.0)
            # alternate ACT/DVE so neither engine eats all 20 copies while
            # also chewing the first block's exp stream
            src = ps[:].rearrange("p (h d) -> p h d", d=64)
            if j % 2 == 0:
                nc.vector.tensor_copy(v1v[:, :, 0:64], src)
            else:
                nc.scalar.copy(v1v[:, :, 0:64], src)

        def scores_part1(h, half, j):
            """s1 half of the scores (cols 0:ACOLS) + the ACT exp -> eta.

            s1 is read only by ACT, s2 only by DVE, so each engine gates only
            its own psum slot chain. ACT is the steady-state pacer, so part1
            is emitted FIRST in each j-body to keep its exps back-to-back.
            """
            m, po = h // 2, (h % 2) * 64
            ps1 = spool.tile([128, ACOLS], F32, tag="s1", name="s1")
            for c0, cw in ((0, 512), (512, 256)):
                nc.tensor.matmul(
                    ps1[:, c0:c0 + cw],
                    lhsT=qk_sb[2 + m][po:po + 64, j * 128:(j + 1) * 128],
                    rhs=qk_sb[m][po:po + 64, half * HALF + c0:half * HALF + c0 + cw],
                    start=True, stop=True,
                )
            eta = epool.tile([128, ACOLS], BF16, tag="ea", name="eta")
            nc.scalar.activation(eta[:], ps1[:], EXP, scale=float(SCALE))
            return eta

        def scores_part2(h, half, j):
            """s2 half of the scores (cols ACOLS:) + the DVE exp -> etb."""
            m, po = h // 2, (h % 2) * 64
            ps2 = spool.tile([128, HALF - ACOLS], F32, tag="s2", name="s2")
            nc.tensor.matmul(
                ps2[:],
                lhsT=qk_sb[2 + m][po:po + 64, j * 128:(j + 1) * 128],
                rhs=qk_sb[m][po:po + 64, half * HALF + ACOLS:(half + 1) * HALF],
                start=True, stop=True,
            )
            etb = epool.tile([128, HALF - ACOLS], BF16, tag="eb", name="etb")
            nc.vector.tensor_scalar(
                etb[:].bitcast(I16), ps2[:],
                float(A_MUL), float(B_ADD),
                mybir.AluOpType.mult, mybir.AluOpType.add)
            return etb

        def attn_mm_half(h, half, ets, i, jlo, jhi, pso, tail=False):
            """Half of out'[i-tile] = sum_j expT_j[:, i].T @ [v|1]."""
            na = ACOLS // 128
            for j in range(jlo, jhi):
                eta, etb = ets[j]
                lhsT = (eta[:, i * 128:(i + 1) * 128] if i < na
                        else etb[:, (i - na) * 128:(i - na + 1) * 128])
                nc.tensor.matmul(
                    pso[:],
                    lhsT=lhsT,
                    rhs=v1s[j][:, h * 65:(h + 1) * 65],
                    start=(j == jlo and jlo == 0), stop=(j == NT - 1),
                )

        def attn_norm(h, half, pso, i):
            rc = sbsm.tile([128, 1], F32, tag="rc", name="rc")
            nc.vector.reciprocal(rc[:], pso[:, 64:65])
            on = sbsm.tile([128, 64], BF16, tag="on", name="on")
            nc.vector.tensor_scalar_mul(on[:], pso[:, 0:64], rc[:])
            return (h, half, on, i)

        def attn_mm(h, half, ets, i, tail=False):
            """out'[i-tile] = sum_j expT_j[:, i].T @ [v|1]; normalize to 'on'.

            The transpose+flush is deferred (attn_fin) so the in-order PE
            queue never waits on the DVE normalize of the same step.
            """
            pool = tails["pool"] if tail else smps
            pso = pool.tile([128, 65], F32, tag="to" if tail else "sm", name="smo")
            attn_mm_half(h, half, ets, i, 0, NT, pso, tail)
            return attn_norm(h, half, pso, i)

        def attn_fin(f, tail=False):
            """PE transpose of 'on' + pair-batched outT flush."""
            h, half, on, i = f
            m, po = h // 2, (h % 2) * 64
            pool = tails["pool"] if tail else smps
            if tail:
                # tail is latency-bound: flush each transpose immediately,
                # on ACT (DVE carries the tail norm + y-copy stream)
                pst = pool.tile([128, 128], BF16, tag="tt", name="smt",
                                bufs=1)
                nc.tensor.transpose(pst[0:64, 0:128], on[:], identb[:])
                ig = half * 10 + i
                nc.vector.tensor_copy(
                    outT[m][po:po + 64, ig * 128:(ig + 1) * 128],
                    pst[0:64, 0:128])
                return
            # pair-batched transposes: even i allocates a [64,256] bf16 psum
            # tile, odd i completes it and flushes both to outT in one copy.
            if i % 2 == 0:
                tstate["grp"] = pool.tile([128, 256], BF16, tag="sm", name="smt")
            pst = tstate["grp"]
            nc.tensor.transpose(pst[0:64, (i % 2) * 128:(i % 2) * 128 + 128],
                                on[:], identb[:])
            if i % 2 == 1:
                ig0 = half * 10 + i - 1
                nc.vector.tensor_copy(
                    outT[m][po:po + 64, ig0 * 128:(ig0 + 2) * 128],
                    pst[0:64, 0:256])

        def y_step(i, tail=False):
            """y[i-tile] = outT[:, i].T @ Wo -> DRAM."""
            psy = (tails["pool"].tile([128, QD], F32, tag="ty", name="smy",
                                       bufs=3)
                   if tail else smps.tile([128, QD], F32, tag="sm", name="smy"))
            for kk in range(2):
                nc.tensor.matmul(
                    psy[:],
                    lhsT=outT[kk][:, i * 128:(i + 1) * 128],
                    rhs=wos[kk][:],
                    start=(kk == 0), stop=(kk == 1),
                )
            ysb = ypool.tile([128, QD], F32, tag="y", name="ysb")
            nc.scalar.copy(ysb[:], psy[:])
            nc.sync.dma_start(y[i * 128:i * 128 + 64, :], ysb[0:64, :])
            nc.gpsimd.dma_start(y[i * 128 + 64:(i + 1) * 128, :], ysb[64:128, :])

        # ---- emission: minimal upfront proj, rest interleaved ----
        # k cols 0:128 + q chunk A unblock the first scores matmul ASAP
        qk_proj(1, 0, 0, chunks=[(0, 128)])
        qk_proj(0, 0, 0)
        qk_proj(1, 0, 0, chunks=[(128, 384)])
        # remaining projection slabs, one CHUNK per slot; slots chosen on
        # even j that do NOT carry a pair-tile allocation (j % 4 == 2 does)
        # so the smps "sm" slot chain never stalls the in-order PE queue
        pend_list = [
            ((0, 0, 0), (1, 0, 0), 4), ((0, 0, 2), (1, 0, 0), 5),
            ((0, 0, 4), (1, 0, 1), 0), ((0, 0, 6), (1, 0, 1), 1),
            ((0, 0, 8), (1, 0, 1), 2), ((0, 0, 10), (0, 0, 1), 0),
            ((0, 0, 12), (0, 0, 1), 1), ((0, 0, 14), (0, 0, 1), 2),
            ((0, 1, 4), (0, 1, 0), 0), ((0, 1, 8), (0, 1, 0), 1),
            ((0, 1, 12), (0, 1, 0), 2), ((0, 1, 16), (1, 1, 0), 0),
            ((1, 0, 0), (1, 1, 0), 1), ((1, 0, 4), (1, 1, 0), 2),
            ((1, 0, 8), (1, 1, 1), 0), ((1, 0, 12), (1, 1, 1), 1),
            ((1, 0, 16), (1, 1, 1), 2), ((1, 1, 0), (0, 1, 1), 0),
            ((1, 1, 4), (0, 1, 1), 1), ((1, 1, 8), (0, 1, 1), 2),
        ]
        pending = {slot: (slab, ci) for slot, slab, ci in pend_list}

        fin_q = []
        prev = None
        for h in range(HPC):
            for half in range(2):
                ets = []
                for j in range(NT):
                    # ready PE work first so the spool-gated scores matmuls
                    # sit last in the in-order PE queue
                    if j % 2 == 0 and fin_q:
                        # (y-steps are all deferred to the tail so the (3,1)
                        # block's ACT stream stays exp-only.)
                        attn_fin(fin_q.pop(0))
                    pr = pending.pop((h, half, j), None)
                    if pr is not None:
                        (ti, m, ph_), ci = pr
                        allc = CHUNKS + [(0, 128), (512, 512), (1024, 256)]
                        qk_proj(ti, m, ph_, chunks=allc[ci:ci + 1])
                    if h == 0 and half == 0:
                        v_proj(j)
                    elif prev is not None and j % 2 == 1:
                        ph, phalf, pets = prev
                        i_mm = j // 2
                        pso = smps.tile([128, 65], F32, tag="sm", name="smo")
                        attn_mm_half(ph, phalf, pets, i_mm, 0, 10, pso)
                        tstate["pso"] = (ph, phalf, pets, pso, i_mm)
                    elif j % 2 == 0 and "pso" in tstate:
                        # complete the previous i's burst (carries the block
                        # it belongs to; at j=0 this is two blocks back)
                        ph, phalf, pets, pso, i_mm = tstate.pop("pso")
                        attn_mm_half(ph, phalf, pets, i_mm, 10, NT, pso)
                        fin_q.append(attn_norm(ph, phalf, pso, i_mm))
                    ets.append((scores_part1(h, half, j),
                                scores_part2(h, half, j)))
                prev = (h, half, ets)
        spool_cm.__exit__(None, None, None)
        tpool = ctx.enter_context(tc.tile_pool(name="tpool", bufs=2, space="PSUM"))
        tails["pool"] = tpool

        def drain_fin(tail):
            f = fin_q.pop(0)
            attn_fin(f, tail=tail)
            if f[0] == 3 and f[1] == 1:
                y_step(10 + f[3], tail=True)

        # complete the half-finished (3,0) i=9 burst, then leftover fins
        if "pso" in tstate:
            ph, phalf, pets, pso, i_mm = tstate.pop("pso")
            attn_mm_half(ph, phalf, pets, i_mm, 10, NT, pso)
            fin_q.append(attn_norm(ph, phalf, pso, i_mm))
        while fin_q:
            drain_fin(False)
        # tail: (3,1) attn pipelined with BOTH y half-streams. i>=6 attn
        # reads only the DVE-produced etb tiles (ready before ACT's last
        # exps), and the deferred first-half y's are ready immediately, so
        # both fill the queue while ACT drains its exp backlog.
        yq = list(range(10))
        for idx, i in enumerate([6, 7, 8, 9, 0, 1, 2, 3, 4, 5]):
            fin_q.append(attn_mm(3, 1, prev[2], i, tail=True))
            if yq:
                y_step(yq.pop(0), tail=True)
            if idx >= 1:
                drain_fin(True)
        while fin_q:
            drain_fin(True)

    nc.compile()
    return nc


def _get_nc():
    if "nc" not in _built:
        _built["nc"] = _build()
    return _built["nc"]


def kernel(x, Wq, Wk, Wv, Wo, bo):
    global last_results
    import ml_dtypes
    x = np.asarray(x, dtype=np.float32)
    Wq = np.asarray(Wq, dtype=np.float32)
    Wk = np.asarray(Wk, dtype=np.float32)
    Wv = np.asarray(Wv, dtype=np.float32)
    Wo = np.asarray(Wo, dtype=np.float32)
    bo = np.asarray(bo, dtype=np.float32)

    nc = _get_nc()
    in_maps = []
    for c in range(8):
        bb, g = divmod(c, 2)
        sl = slice(g * IPC, (g + 1) * IPC)
        in_maps.append({
            "xT": np.ascontiguousarray(x[bb].T).astype(ml_dtypes.bfloat16),
            "wq": np.ascontiguousarray(Wq[:, sl]).astype(ml_dtypes.bfloat16),
            "wk": np.ascontiguousarray(Wk[:, sl]).astype(ml_dtypes.bfloat16),
            "wv": np.ascontiguousarray(Wv[:, sl]).astype(ml_dtypes.bfloat16),
            "wo": np.ascontiguousarray(Wo[sl, :]).astype(ml_dtypes.bfloat16),
        })
    res = run_bass_kernel_spmd(nc, in_maps, core_ids=list(range(8)))
    last_results = res
    parts = [r["y"] for r in res.results]
    out = np.empty((B, N, QD), dtype=np.float32)
    for bb in range(B):
        out[bb] = parts[2 * bb] + parts[2 * bb + 1]
    out += bo
    return out


# revision 7
# speedup vs baseline: 1.0384x; 1.0022x over previous
"""Fused multi-head self-attention kernel for Trainium2 (Bass/Tile). v2.

Problem: x:[4,2560,320] f32, Wq/Wk/Wv:[320,512], Wo:[512,320], bo:[320]
  q,k,v = x@W*, 8 heads x 64; sim = q k^T * d^-0.5; attn = softmax(sim);
  out = (attn @ v) @ Wo + bo.

Sharding: batch*head 32-way -> 8 cores: core c handles batch c//2 and the
4-head group c%2. Host sums the two half-head partial output projections
per batch and adds the bias.

Per-core layout trick: scores are computed TRANSPOSED (sT[j,i] = k_j . q_i)
so that the softmax denominator arrives for free: v gets a ones-column
appended, and out' = expT_slice.T @ [v|1] accumulates both attn@v and the
row sums. Normalisation is a per-partition reciprocal+multiply.

Engine budget (cost model, per core): PE ~160us is the floor (scores
204.8k cyc + attn@v 104k + projections/transposes ~69k). The ACT-engine
exp over 26M elements (~218us alone -- the old bottleneck) is split:
 - ACT computes exp on score cols [0:ACOLS] natively; DVE computes cols
   [ACOLS:1280] with a one-instruction Schraudolph bitcast exp
   (int16(x*A+B) viewed as bf16; ~1.8% rms there, 40% of weights).
 - eta/etb live in SEPARATE tiles and the scores land in two psum tiles
   (s1 ACT-read / s2 DVE-read), so neither engine's slot chain gates the
   other (shared tiles serialize via write-write/read hazards).
 - x/Wq/Wk/Wv/Wo are bf16 (host-side cast): halves DMA + SBUF footprint.
 - attn normalize ('on') is bf16 so PE transposes run at 1 cyc/row; outT
   is bf16; transposes flushed in pairs; attn transpose+flush deferred one
   step so the in-order PE queue never waits on the DVE normalize.
 - all y-steps run in the tail (keeps the last block's ACT stream
   exp-only); tail attn runs i>=6 first (needs only DVE-produced etb).
 - each attn i-tile's 20-matmul psum accumulation is split 14/6 across
   two j-slots, smoothing PE load so transient exp lag never stalls it.
"""

import sys

import numpy as np

if "/opt/trn_rl_repo" not in sys.path:
    sys.path.insert(0, "/opt/trn_rl_repo")

from contextlib import ExitStack

import concourse.bass as bass
from concourse import bacc
import concourse.mybir as mybir
import concourse.tile as tile
from concourse.bass_utils import run_bass_kernel_spmd
from concourse.masks import make_identity

# ---- problem constants (hardcoded per contract) ----
B = 4
N = 2560
QD = 320
H_TOT = 8
D = 64
HPC = 4                  # heads per core
IPC = HPC * D            # 256 inner dims per core
SCALE = D ** -0.5
NT = N // 128            # 20 n-tiles
HALF = N // 2            # 1280
F32 = mybir.dt.float32
BF16 = mybir.dt.bfloat16
F32R = mybir.dt.float32r
I16 = mybir.dt.int16
EXP = mybir.ActivationFunctionType.Exp

# qd (=320) split into K subtiles for the 128-partition contraction
KS = [(0, 128), (128, 128), (256, 64)]
# 1280-wide column chunks (PSUM-bank-aligned matmul N<=512)
CHUNKS = [(0, 512), (512, 512), (1024, 256)]

# exp split: ACT native exp on [0:ACOLS], DVE Schraudolph on [ACOLS:HALF].
# ACOLS must be a multiple of 128 so the two exp outputs can live in
# SEPARATE tiles (shared-tile writes serialize ACT behind DVE via a
# write-write hazard) with attnv i-tiles reading wholly from one of them.
ACOLS = 768
A_MUL = 128.0 * SCALE / np.log(2.0)       # folds the 1/sqrt(d) scale
B_ADD = 128.0 * (127.0 - 0.058)           # rounding-convert-optimal bias

EXP_BUFS = 43            # expT pool slots of [128,1280] bf16 (2.5KB/part each)

_built = {}
last_results = None      # stashed BassKernelResults for the test harness


def _build():
    nc = bacc.Bacc(None, target_bir_lowering=False)
    xT = nc.declare_dram_parameter("xT", [QD, N], BF16, isOutput=False)
    wq = nc.declare_dram_parameter("wq", [QD, IPC], BF16, isOutput=False)
    wk = nc.declare_dram_parameter("wk", [QD, IPC], BF16, isOutput=False)
    wv = nc.declare_dram_parameter("wv", [QD, IPC], BF16, isOutput=False)
    wo = nc.declare_dram_parameter("wo", [IPC, QD], BF16, isOutput=False)
    y = nc.declare_dram_parameter("y", [N, QD], F32, isOutput=True)

    with tile.TileContext(nc) as tc, ExitStack() as ctx:
        const = ctx.enter_context(tc.tile_pool(name="const", bufs=1))
        smps = ctx.enter_context(tc.tile_pool(name="smps", bufs=2, space="PSUM"))
        epool = ctx.enter_context(tc.tile_pool(name="epool", bufs=EXP_BUFS))
        sbsm = ctx.enter_context(tc.tile_pool(name="sbsm", bufs=4))
        ypool = ctx.enter_context(tc.tile_pool(name="ypool", bufs=5))
        spool_cm = tc.tile_pool(name="spool", bufs=2, space="PSUM")
        spool = spool_cm.__enter__()

        ident = const.tile([128, 128], F32, tag="ident", name="ident")
        make_identity(nc, ident[:])
        identb = const.tile([128, 128], BF16, tag="identb", name="identb")
        nc.vector.tensor_copy(identb[:], ident[:])
        warm = sbsm.tile([128, 1], F32, tag="rc", name="warm")
        nc.scalar.activation(warm[:], ident[:, 0:1], EXP, scale=1.0)
        for _ in range(6):
            pw = smps.tile([128, 128], F32, tag="sm", name="pwarm")
            nc.tensor.matmul(pw[:], lhsT=ident[:], rhs=ident[:],
                             start=True, stop=True)

        # ---- persistent inputs (DMA emission ordered by first use) ----
        xts = [const.tile([128, N], BF16, tag=f"xt{ki}", name=f"xt{ki}")
               for ki in range(3)]
        wqs = [const.tile([128, IPC], BF16, tag=f"wq{ki}", name=f"wq{ki}")
               for ki in range(3)]
        wks = [const.tile([128, IPC], BF16, tag=f"wk{ki}", name=f"wk{ki}")
               for ki in range(3)]
        wvs = [const.tile([128, IPC], BF16, tag=f"wv{ki}", name=f"wv{ki}")
               for ki in range(3)]
        wos = [const.tile([128, QD], BF16, tag=f"wo{kk}", name=f"wo{kk}")
               for kk in range(2)]
        # critical set first; x split across BOTH dma queues (sync + gpsimd)
        for ki, (k0, kw) in enumerate(KS):
            nc.sync.dma_start(xts[ki][:kw, 0:640], xT[k0:k0 + kw, 0:640])
            nc.gpsimd.dma_start(wqs[ki][:kw, :], wq[k0:k0 + kw, :])
        for ki, (k0, kw) in enumerate(KS):
            nc.sync.dma_start(xts[ki][:kw, 640:1280], xT[k0:k0 + kw, 640:1280])
            nc.gpsimd.dma_start(wks[ki][:kw, :], wk[k0:k0 + kw, :])
        for ki, (k0, kw) in enumerate(KS):
            nc.gpsimd.dma_start(xts[ki][:kw, 1280:1920], xT[k0:k0 + kw, 1280:1920])
            nc.sync.dma_start(xts[ki][:kw, 1920:2560], xT[k0:k0 + kw, 1920:2560])
        for ki, (k0, kw) in enumerate(KS):
            nc.gpsimd.dma_start(wvs[ki][:kw, :], wv[k0:k0 + kw, :])
        for kk in range(2):
            nc.gpsimd.dma_start(wos[kk][:], wo[kk * 128:(kk + 1) * 128, :])

        # qT/kT: [inner(256) x n] as 2 tiles of [128, N] each; fp32 storage
        qk_sb = [const.tile([128, N], F32R, tag=f"qk{i}", name=f"qk{i}") for i in range(4)]
        # outT: normalized attention output, [inner x n], bf16
        outT = [const.tile([128, N], BF16, tag=f"oT{kk}", name=f"oT{kk}") for kk in range(2)]
        # v with ones column per head: [n-tile][128, 4*65] bf16
        v1s = [const.tile([128, HPC * 65], BF16, tag=f"v1_{j}", name=f"v1_{j}") for j in range(NT)]

        ws = [wqs, wks]
        tails = {}
        tstate = {}

        def qk_proj(ti, m, half, chunks=None):
            """qT/kT tile ti(0=q,1=k), inner slab m, col half -> qk_sb[ti*2+m].

            PSUM->SBUF copies: 512-wide chunks go to ACT (has startup slack),
            the 256-wide chunk to DVE, to keep early-block DVE load down.
            """
            for c0, cw in (chunks or CHUNKS):
                ps = smps.tile([128, 512], F32, tag="sm", name="smp")
                for ki, (k0, kw) in enumerate(KS):
                    nc.tensor.matmul(
                        ps[:, 0:cw],
                        lhsT=ws[ti][ki][:kw, m * 128:(m + 1) * 128],
                        rhs=xts[ki][:kw, half * HALF + c0:half * HALF + c0 + cw],
                        start=(ki == 0), stop=(ki == 2),
                    )
                dst = qk_sb[ti * 2 + m][:, half * HALF + c0:half * HALF + c0 + cw]
                if c0 == 0:
                    nc.scalar.copy(dst, ps[:, 0:cw])
                else:
                    nc.vector.tensor_copy(dst, ps[:, 0:cw])

        def v_proj(j):
            """v for n-tile j (all 4 heads) -> v1s[j] bf16 with ones cols."""
            ps = smps.tile([128, IPC], F32, tag="sm", name="smv")
            for ki, (k0, kw) in enumerate(KS):
                nc.tensor.matmul(
                    ps[:],
                    lhsT=xts[ki][:kw, j * 128:(j + 1) * 128],
                    rhs=wvs[ki][:kw, :],
                    start=(ki == 0), stop=(ki == 2),
                )
            v1v = v1s[j][:].rearrange("p (h e) -> p h e", e=65)
            nc.gpsimd.memset(v1v[:, :, 64:65], 1.0)
            # alternate ACT/DVE so neither engine eats all 20 copies while
            # also chewing the first block's exp stream
            src = ps[:].rearrange("p (h d) -> p h d", d=64)
            if j % 2 == 0:
                nc.vector.tensor_copy(v1v[:, :, 0:64], src)
            else:
                nc.scalar.copy(v1v[:, :, 0:64], src)

        def scores_part1(h, half, j):
            """s1 half of the scores (cols 0:ACOLS) + the ACT exp -> eta.

            s1 is read only by ACT, s2 only by DVE, so each engine gates only
            its own psum slot chain. ACT is the steady-state pacer, so part1
            is emitted FIRST in each j-body to keep its exps back-to-back.
            """
            m, po = h // 2, (h % 2) * 64
            ps1 = spool.tile([128, ACOLS], F32, tag="s1", name="s1")
            for c0, cw in ((0, 512), (512, 256)):
                nc.tensor.matmul(
                    ps1[:, c0:c0 + cw],
                    lhsT=qk_sb[2 + m][po:po + 64, j * 128:(j + 1) * 128],
                    rhs=qk_sb[m][po:po + 64, half * HALF + c0:half * HALF + c0 + cw],
                    start=True, stop=True,
                )
            eta = epool.tile([128, ACOLS], BF16, tag="ea", name="eta")
            nc.scalar.activation(eta[:], ps1[:], EXP, scale=float(SCALE))
            return eta

        def scores_part2(h, half, j):
            """s2 half of the scores (cols ACOLS:) + the DVE exp -> etb."""
            m, po = h // 2, (h % 2) * 64
            ps2 = spool.tile([128, HALF - ACOLS], F32, tag="s2", name="s2")
            nc.tensor.matmul(
                ps2[:],
                lhsT=qk_sb[2 + m][po:po + 64, j * 128:(j + 1) * 128],
                rhs=qk_sb[m][po:po + 64, half * HALF + ACOLS:(half + 1) * HALF],
                start=True, stop=True,
            )
            etb = epool.tile([128, HALF - ACOLS], BF16, tag="eb", name="etb")
            nc.vector.tensor_scalar(
                etb[:].bitcast(I16), ps2[:],
                float(A_MUL), float(B_ADD),
                mybir.AluOpType.mult, mybir.AluOpType.add)
            return etb

        def attn_mm_half(h, half, ets, i, jlo, jhi, pso, tail=False):
            """Half of out'[i-tile] = sum_j expT_j[:, i].T @ [v|1]."""
            na = ACOLS // 128
            for j in range(jlo, jhi):
                eta, etb = ets[j]
                lhsT = (eta[:, i * 128:(i + 1) * 128] if i < na
                        else etb[:, (i - na) * 128:(i - na + 1) * 128])
                nc.tensor.matmul(
                    pso[:],
                    lhsT=lhsT,
                    rhs=v1s[j][:, h * 65:(h + 1) * 65],
                    start=(j == jlo and jlo == 0), stop=(j == NT - 1),
                )

        def attn_norm(h, half, pso, i):
            rc = sbsm.tile([128, 1], F32, tag="rc", name="rc")
            nc.vector.reciprocal(rc[:], pso[:, 64:65])
            on = sbsm.tile([128, 64], BF16, tag="on", name="on")
            nc.vector.tensor_scalar_mul(on[:], pso[:, 0:64], rc[:])
            return (h, half, on, i)

        def attn_mm(h, half, ets, i, tail=False):
            """out'[i-tile] = sum_j expT_j[:, i].T @ [v|1]; normalize to 'on'.

            The transpose+flush is deferred (attn_fin) so the in-order PE
            queue never waits on the DVE normalize of the same step.
            """
            pool = tails["pool"] if tail else smps
            pso = pool.tile([128, 65], F32, tag="to" if tail else "sm", name="smo")
            attn_mm_half(h, half, ets, i, 0, NT, pso, tail)
            return attn_norm(h, half, pso, i)

        def attn_fin(f, tail=False):
            """PE transpose of 'on' + pair-batched outT flush."""
            h, half, on, i = f
            m, po = h // 2, (h % 2) * 64
            pool = tails["pool"] if tail else smps
            if tail:
                # tail is latency-bound: flush each transpose immediately,
                # on ACT (DVE carries the tail norm + y-copy stream)
                pst = pool.tile([128, 128], BF16, tag="tt", name="smt",
                                bufs=1)
                nc.tensor.transpose(pst[0:64, 0:128], on[:], identb[:])
                ig = half * 10 + i
                nc.vector.tensor_copy(
                    outT[m][po:po + 64, ig * 128:(ig + 1) * 128],
                    pst[0:64, 0:128])
                return
            # pair-batched transposes: even i allocates a [64,256] bf16 psum
            # tile, odd i completes it and flushes both to outT in one copy.
            if i % 2 == 0:
                tstate["grp"] = pool.tile([128, 256], BF16, tag="sm", name="smt")
            pst = tstate["grp"]
            nc.tensor.transpose(pst[0:64, (i % 2) * 128:(i % 2) * 128 + 128],
                                on[:], identb[:])
            if i % 2 == 1:
                ig0 = half * 10 + i - 1
                nc.vector.tensor_copy(
                    outT[m][po:po + 64, ig0 * 128:(ig0 + 2) * 128],
                    pst[0:64, 0:256])

        def y_step(i, tail=False):
            """y[i-tile] = outT[:, i].T @ Wo -> DRAM."""
            psy = (tails["pool"].tile([128, QD], F32, tag="ty", name="smy",
                                       bufs=3)
                   if tail else smps.tile([128, QD], F32, tag="sm", name="smy"))
            for kk in range(2):
                nc.tensor.matmul(
                    psy[:],
                    lhsT=outT[kk][:, i * 128:(i + 1) * 128],
                    rhs=wos[kk][:],
                    start=(kk == 0), stop=(kk == 1),
                )
            ysb = ypool.tile([128, QD], F32, tag="y", name="ysb")
            nc.scalar.copy(ysb[:], psy[:])
            nc.sync.dma_start(y[i * 128:i * 128 + 64, :], ysb[0:64, :])
            nc.gpsimd.dma_start(y[i * 128 + 64:(i + 1) * 128, :], ysb[64:128, :])

        # ---- emission: minimal upfront proj, rest interleaved ----
        # k cols 0:128 + q chunk A unblock the first scores matmul ASAP
        qk_proj(1, 0, 0, chunks=[(0, 128)])
        qk_proj(0, 0, 0)
        qk_proj(1, 0, 0, chunks=[(128, 384)])
        # remaining projection slabs, one CHUNK per slot; slots chosen on
        # even j that do NOT carry a pair-tile allocation (j % 4 == 2 does)
        # so the smps "sm" slot chain never stalls the in-order PE queue
        pend_list = [
            ((0, 0, 0), (1, 0, 0), 4), ((0, 0, 2), (1, 0, 0), 5),
            ((0, 0, 4), (1, 0, 1), 0), ((0, 0, 6), (1, 0, 1), 1),
            ((0, 0, 8), (1, 0, 1), 2), ((0, 0, 10), (0, 0, 1), 0),
            ((0, 0, 12), (0, 0, 1), 1), ((0, 0, 14), (0, 0, 1), 2),
            ((0, 1, 4), (0, 1, 0), 0), ((0, 1, 8), (0, 1, 0), 1),
            ((0, 1, 12), (0, 1, 0), 2), ((0, 1, 16), (1, 1, 0), 0),
            ((1, 0, 0), (1, 1, 0), 1), ((1, 0, 4), (1, 1, 0), 2),
            ((1, 0, 8), (1, 1, 1), 0), ((1, 0, 12), (1, 1, 1), 1),
            ((1, 0, 16), (1, 1, 1), 2), ((1, 1, 0), (0, 1, 1), 0),
            ((1, 1, 4), (0, 1, 1), 1), ((1, 1, 8), (0, 1, 1), 2),
        ]
        pending = {slot: (slab, ci) for slot, slab, ci in pend_list}

        fin_q = []
        prev = None
        for h in range(HPC):
            for half in range(2):
                ets = []
                for j in range(NT):
                    # ready PE work first so the spool-gated scores matmuls
                    # sit last in the in-order PE queue
                    if j % 2 == 0 and fin_q:
                        # (y-steps are all deferred to the tail so the (3,1)
                        # block's ACT stream stays exp-only.)
                        attn_fin(fin_q.pop(0))
                    pr = pending.pop((h, half, j), None)
                    if pr is not None:
                        (ti, m, ph_), ci = pr
                        allc = CHUNKS + [(0, 128), (512, 512), (1024, 256)]
                        qk_proj(ti, m, ph_, chunks=allc[ci:ci + 1])
                    if h == 0 and half == 0:
                        v_proj(j)
                    elif prev is not None and j % 2 == 1:
                        ph, phalf, pets = prev
                        i_mm = j // 2
                        pso = smps.tile([128, 65], F32, tag="sm", name="smo")
                        attn_mm_half(ph, phalf, pets, i_mm, 0, 14, pso)
                        tstate["pso"] = (ph, phalf, pets, pso, i_mm)
                    elif j % 2 == 0 and "pso" in tstate:
                        # complete the previous i's burst (carries the block
                        # it belongs to; at j=0 this is two blocks back)
                        ph, phalf, pets, pso, i_mm = tstate.pop("pso")
                        attn_mm_half(ph, phalf, pets, i_mm, 14, NT, pso)
                        fin_q.append(attn_norm(ph, phalf, pso, i_mm))
                    ets.append((scores_part1(h, half, j),
                                scores_part2(h, half, j)))
                prev = (h, half, ets)
        spool_cm.__exit__(None, None, None)
        tpool = ctx.enter_context(tc.tile_pool(name="tpool", bufs=2, space="PSUM"))
        tails["pool"] = tpool

        def drain_fin(tail):
            f = fin_q.pop(0)
            attn_fin(f, tail=tail)
            if f[0] == 3 and f[1] == 1:
                y_step(10 + f[3], tail=True)

        # complete the half-finished (3,0) i=9 burst, then leftover fins
        if "pso" in tstate:
            ph, phalf, pets, pso, i_mm = tstate.pop("pso")
            attn_mm_half(ph, phalf, pets, i_mm, 14, NT, pso)
            fin_q.append(attn_norm(ph, phalf, pso, i_mm))
        while fin_q:
            drain_fin(False)
        # tail: (3,1) attn pipelined with BOTH y half-streams. i>=6 attn
        # reads only the DVE-produced etb tiles (ready before ACT's last
        # exps), and the deferred first-half y's are ready immediately, so
        # both fill the queue while ACT drains its exp backlog.
        yq = list(range(10))
        for idx, i in enumerate([6, 7, 8, 9, 0, 1, 2, 3, 4, 5]):
            fin_q.append(attn_mm(3, 1, prev[2], i, tail=True))
            if yq:
                y_step(yq.pop(0), tail=True)
            if idx >= 1:
                drain_fin(True)
        while fin_q:
            drain_fin(True)

    nc.compile()
    return nc


def _get_nc():
    if "nc" not in _built:
        _built["nc"] = _build()
    return _built["nc"]


def kernel(x, Wq, Wk, Wv, Wo, bo):
    global last_results
    import ml_dtypes
    x = np.asarray(x, dtype=np.float32)
    Wq = np.asarray(Wq, dtype=np.float32)
    Wk = np.asarray(Wk, dtype=np.float32)
    Wv = np.asarray(Wv, dtype=np.float32)
    Wo = np.asarray(Wo, dtype=np.float32)
    bo = np.asarray(bo, dtype=np.float32)

    nc = _get_nc()
    in_maps = []
    for c in range(8):
        bb, g = divmod(c, 2)
        sl = slice(g * IPC, (g + 1) * IPC)
        in_maps.append({
            "xT": np.ascontiguousarray(x[bb].T).astype(ml_dtypes.bfloat16),
            "wq": np.ascontiguousarray(Wq[:, sl]).astype(ml_dtypes.bfloat16),
            "wk": np.ascontiguousarray(Wk[:, sl]).astype(ml_dtypes.bfloat16),
            "wv": np.ascontiguousarray(Wv[:, sl]).astype(ml_dtypes.bfloat16),
            "wo": np.ascontiguousarray(Wo[sl, :]).astype(ml_dtypes.bfloat16),
        })
    res = run_bass_kernel_spmd(nc, in_maps, core_ids=list(range(8)))
    last_results = res
    parts = [r["y"] for r in res.results]
    out = np.empty((B, N, QD), dtype=np.float32)
    for bb in range(B):
        out[bb] = parts[2 * bb] + parts[2 * bb + 1]
    out += bo
    return out


# revision 8
# speedup vs baseline: 1.0529x; 1.0140x over previous
"""Fused multi-head self-attention kernel for Trainium2 (Bass/Tile). v2.

Problem: x:[4,2560,320] f32, Wq/Wk/Wv:[320,512], Wo:[512,320], bo:[320]
  q,k,v = x@W*, 8 heads x 64; sim = q k^T * d^-0.5; attn = softmax(sim);
  out = (attn @ v) @ Wo + bo.

Sharding: batch*head 32-way -> 8 cores: core c handles batch c//2 and the
4-head group c%2. Host sums the two half-head partial output projections
per batch and adds the bias.

Per-core layout trick: scores are computed TRANSPOSED (sT[j,i] = k_j . q_i)
so that the softmax denominator arrives for free: v gets a ones-column
appended, and out' = expT_slice.T @ [v|1] accumulates both attn@v and the
row sums. Normalisation is a per-partition reciprocal+multiply.

Engine budget (cost model, per core): PE ~160us is the floor (scores
204.8k cyc + attn@v 104k + projections/transposes ~69k). The ACT-engine
exp over 26M elements (~218us alone -- the old bottleneck) is split:
 - ACT computes exp on score cols [0:ACOLS] natively; DVE computes cols
   [ACOLS:1280] with a one-instruction Schraudolph bitcast exp
   (int16(x*A+B) viewed as bf16; ~1.8% rms there, 40% of weights).
 - eta/etb live in SEPARATE tiles and the scores land in two psum tiles
   (s1 ACT-read / s2 DVE-read), so neither engine's slot chain gates the
   other (shared tiles serialize via write-write/read hazards).
 - x/Wq/Wk/Wv/Wo are bf16 (host-side cast): halves DMA + SBUF footprint.
 - attn normalize ('on') is bf16 so PE transposes run at 1 cyc/row; outT
   is bf16; transposes flushed in pairs; attn transpose+flush deferred one
   step so the in-order PE queue never waits on the DVE normalize.
 - all y-steps run in the tail (keeps the last block's ACT stream
   exp-only); tail attn runs i>=6 first (needs only DVE-produced etb).
"""

import sys

import numpy as np

if "/opt/trn_rl_repo" not in sys.path:
    sys.path.insert(0, "/opt/trn_rl_repo")

from contextlib import ExitStack

import concourse.bass as bass
from concourse import bacc
import concourse.mybir as mybir
import concourse.tile as tile
from concourse.bass_utils import run_bass_kernel_spmd
from concourse.masks import make_identity

# ---- problem constants (hardcoded per contract) ----
B = 4
N = 2560
QD = 320
H_TOT = 8
D = 64
HPC = 4                  # heads per core
IPC = HPC * D            # 256 inner dims per core
SCALE = D ** -0.5
NT = N // 128            # 20 n-tiles
HALF = N // 2            # 1280
F32 = mybir.dt.float32
BF16 = mybir.dt.bfloat16
F32R = mybir.dt.float32r
I16 = mybir.dt.int16
EXP = mybir.ActivationFunctionType.Exp

# qd (=320) split into K subtiles for the 128-partition contraction
KS = [(0, 128), (128, 128), (256, 64)]
# 1280-wide column chunks (PSUM-bank-aligned matmul N<=512)
CHUNKS = [(0, 512), (512, 512), (1024, 256)]

# exp split: ACT native exp on [0:ACOLS], DVE Schraudolph on [ACOLS:HALF].
# ACOLS must be a multiple of 128 so the two exp outputs can live in
# SEPARATE tiles (shared-tile writes serialize ACT behind DVE via a
# write-write hazard) with attnv i-tiles reading wholly from one of them.
ACOLS = 768
A_MUL = 128.0 * SCALE / np.log(2.0)       # folds the 1/sqrt(d) scale
B_ADD = 128.0 * (127.0 - 0.058)           # rounding-convert-optimal bias

EXP_BUFS = 43            # expT pool slots of [128,1280] bf16 (2.5KB/part each)

_built = {}
last_results = None      # stashed BassKernelResults for the test harness


def _build():
    nc = bacc.Bacc(None, target_bir_lowering=False)
    xT = nc.declare_dram_parameter("xT", [QD, N], BF16, isOutput=False)
    wq = nc.declare_dram_parameter("wq", [QD, IPC], BF16, isOutput=False)
    wk = nc.declare_dram_parameter("wk", [QD, IPC], BF16, isOutput=False)
    wv = nc.declare_dram_parameter("wv", [QD, IPC], BF16, isOutput=False)
    wo = nc.declare_dram_parameter("wo", [IPC, QD], BF16, isOutput=False)
    y = nc.declare_dram_parameter("y", [N, QD], F32, isOutput=True)

    with tile.TileContext(nc) as tc, ExitStack() as ctx:
        const = ctx.enter_context(tc.tile_pool(name="const", bufs=1))
        smps = ctx.enter_context(tc.tile_pool(name="smps", bufs=2, space="PSUM"))
        epool = ctx.enter_context(tc.tile_pool(name="epool", bufs=EXP_BUFS))
        sbsm = ctx.enter_context(tc.tile_pool(name="sbsm", bufs=4))
        ypool = ctx.enter_context(tc.tile_pool(name="ypool", bufs=5))
        spool_cm = tc.tile_pool(name="spool", bufs=2, space="PSUM")
        spool = spool_cm.__enter__()

        ident = const.tile([128, 128], F32, tag="ident", name="ident")
        make_identity(nc, ident[:])
        identb = const.tile([128, 128], BF16, tag="identb", name="identb")
        nc.vector.tensor_copy(identb[:], ident[:])
        warm = sbsm.tile([128, 1], F32, tag="rc", name="warm")
        nc.scalar.activation(warm[:], ident[:, 0:1], EXP, scale=1.0)
        for _ in range(4):
            pw = smps.tile([128, 128], F32, tag="sm", name="pwarm")
            nc.tensor.matmul(pw[:], lhsT=ident[:], rhs=ident[:],
                             start=True, stop=True)

        # ---- persistent inputs (DMA emission ordered by first use) ----
        xts = [const.tile([128, N], BF16, tag=f"xt{ki}", name=f"xt{ki}")
               for ki in range(3)]
        wqs = [const.tile([128, IPC], BF16, tag=f"wq{ki}", name=f"wq{ki}")
               for ki in range(3)]
        wks = [const.tile([128, IPC], BF16, tag=f"wk{ki}", name=f"wk{ki}")
               for ki in range(3)]
        wvs = [const.tile([128, IPC], BF16, tag=f"wv{ki}", name=f"wv{ki}")
               for ki in range(3)]
        wos = [const.tile([128, QD], BF16, tag=f"wo{kk}", name=f"wo{kk}")
               for kk in range(2)]
        # critical set first; x split across BOTH dma queues (sync + gpsimd)
        for ki, (k0, kw) in enumerate(KS):
            nc.sync.dma_start(xts[ki][:kw, 0:640], xT[k0:k0 + kw, 0:640])
            nc.gpsimd.dma_start(wqs[ki][:kw, :], wq[k0:k0 + kw, :])
        for ki, (k0, kw) in enumerate(KS):
            nc.sync.dma_start(xts[ki][:kw, 640:1280], xT[k0:k0 + kw, 640:1280])
            nc.gpsimd.dma_start(wks[ki][:kw, :], wk[k0:k0 + kw, :])
        for ki, (k0, kw) in enumerate(KS):
            nc.gpsimd.dma_start(xts[ki][:kw, 1280:1920], xT[k0:k0 + kw, 1280:1920])
            nc.sync.dma_start(xts[ki][:kw, 1920:2560], xT[k0:k0 + kw, 1920:2560])
        for ki, (k0, kw) in enumerate(KS):
            nc.gpsimd.dma_start(wvs[ki][:kw, :], wv[k0:k0 + kw, :])
        for kk in range(2):
            nc.gpsimd.dma_start(wos[kk][:], wo[kk * 128:(kk + 1) * 128, :])

        # qT/kT: [inner(256) x n] as 2 tiles of [128, N] each; fp32 storage
        qk_sb = [const.tile([128, N], F32R, tag=f"qk{i}", name=f"qk{i}") for i in range(4)]
        # outT: normalized attention output, [inner x n], bf16
        outT = [const.tile([128, N], BF16, tag=f"oT{kk}", name=f"oT{kk}") for kk in range(2)]
        # v with ones column per head: [n-tile][128, 4*65] bf16
        v1s = [const.tile([128, HPC * 65], BF16, tag=f"v1_{j}", name=f"v1_{j}") for j in range(NT)]

        ws = [wqs, wks]
        tails = {}
        tstate = {}

        def qk_proj(ti, m, half, chunks=None):
            """qT/kT tile ti(0=q,1=k), inner slab m, col half -> qk_sb[ti*2+m].

            PSUM->SBUF copies: 512-wide chunks go to ACT (has startup slack),
            the 256-wide chunk to DVE, to keep early-block DVE load down.
            """
            for c0, cw in (chunks or CHUNKS):
                ps = smps.tile([128, 512], F32, tag="sm", name="smp")
                for ki, (k0, kw) in enumerate(KS):
                    nc.tensor.matmul(
                        ps[:, 0:cw],
                        lhsT=ws[ti][ki][:kw, m * 128:(m + 1) * 128],
                        rhs=xts[ki][:kw, half * HALF + c0:half * HALF + c0 + cw],
                        start=(ki == 0), stop=(ki == 2),
                    )
                dst = qk_sb[ti * 2 + m][:, half * HALF + c0:half * HALF + c0 + cw]
                if c0 == 0:
                    nc.scalar.copy(dst, ps[:, 0:cw])
                else:
                    nc.vector.tensor_copy(dst, ps[:, 0:cw])

        def v_proj(j):
            """v for n-tile j (all 4 heads) -> v1s[j] bf16 with ones cols."""
            ps = smps.tile([128, IPC], F32, tag="sm", name="smv")
            for ki, (k0, kw) in enumerate(KS):
                nc.tensor.matmul(
                    ps[:],
                    lhsT=xts[ki][:kw, j * 128:(j + 1) * 128],
                    rhs=wvs[ki][:kw, :],
                    start=(ki == 0), stop=(ki == 2),
                )
            v1v = v1s[j][:].rearrange("p (h e) -> p h e", e=65)
            nc.gpsimd.memset(v1v[:, :, 64:65], 1.0)
            # alternate ACT/DVE so neither engine eats all 20 copies while
            # also chewing the first block's exp stream
            src = ps[:].rearrange("p (h d) -> p h d", d=64)
            if j % 2 == 0:
                nc.vector.tensor_copy(v1v[:, :, 0:64], src)
            else:
                nc.scalar.copy(v1v[:, :, 0:64], src)

        def scores_part1(h, half, j):
            """s1 half of the scores (cols 0:ACOLS) + the ACT exp -> eta.

            s1 is read only by ACT, s2 only by DVE, so each engine gates only
            its own psum slot chain. ACT is the steady-state pacer, so part1
            is emitted FIRST in each j-body to keep its exps back-to-back.
            """
            m, po = h // 2, (h % 2) * 64
            ps1 = spool.tile([128, ACOLS], F32, tag="s1", name="s1")
            for c0, cw in ((0, 512), (512, 256)):
                nc.tensor.matmul(
                    ps1[:, c0:c0 + cw],
                    lhsT=qk_sb[2 + m][po:po + 64, j * 128:(j + 1) * 128],
                    rhs=qk_sb[m][po:po + 64, half * HALF + c0:half * HALF + c0 + cw],
                    start=True, stop=True,
                )
            eta = epool.tile([128, ACOLS], BF16, tag="ea", name="eta")
            nc.scalar.activation(eta[:], ps1[:], EXP, scale=float(SCALE))
            return eta

        def scores_part2(h, half, j):
            """s2 half of the scores (cols ACOLS:) + the DVE exp -> etb."""
            m, po = h // 2, (h % 2) * 64
            ps2 = spool.tile([128, HALF - ACOLS], F32, tag="s2", name="s2")
            nc.tensor.matmul(
                ps2[:],
                lhsT=qk_sb[2 + m][po:po + 64, j * 128:(j + 1) * 128],
                rhs=qk_sb[m][po:po + 64, half * HALF + ACOLS:(half + 1) * HALF],
                start=True, stop=True,
            )
            etb = epool.tile([128, HALF - ACOLS], BF16, tag="eb", name="etb")
            nc.vector.tensor_scalar(
                etb[:].bitcast(I16), ps2[:],
                float(A_MUL), float(B_ADD),
                mybir.AluOpType.mult, mybir.AluOpType.add)
            return etb

        def attn_mm_half(h, half, ets, i, jlo, jhi, pso, tail=False):
            """Half of out'[i-tile] = sum_j expT_j[:, i].T @ [v|1]."""
            na = ACOLS // 128
            for j in range(jlo, jhi):
                eta, etb = ets[j]
                lhsT = (eta[:, i * 128:(i + 1) * 128] if i < na
                        else etb[:, (i - na) * 128:(i - na + 1) * 128])
                nc.tensor.matmul(
                    pso[:],
                    lhsT=lhsT,
                    rhs=v1s[j][:, h * 65:(h + 1) * 65],
                    start=(j == jlo and jlo == 0), stop=(j == NT - 1),
                )

        def attn_norm(h, half, pso, i):
            rc = sbsm.tile([128, 1], F32, tag="rc", name="rc")
            nc.vector.reciprocal(rc[:], pso[:, 64:65])
            on = sbsm.tile([128, 64], BF16, tag="on", name="on")
            nc.vector.tensor_scalar_mul(on[:], pso[:, 0:64], rc[:])
            return (h, half, on, i)

        def attn_mm(h, half, ets, i, tail=False):
            """out'[i-tile] = sum_j expT_j[:, i].T @ [v|1]; normalize to 'on'.

            The transpose+flush is deferred (attn_fin) so the in-order PE
            queue never waits on the DVE normalize of the same step.
            """
            pool = tails["pool"] if tail else smps
            pso = pool.tile([128, 65], F32, tag="to" if tail else "sm", name="smo")
            attn_mm_half(h, half, ets, i, 0, NT, pso, tail)
            return attn_norm(h, half, pso, i)

        def attn_fin(f, tail=False):
            """PE transpose of 'on' + pair-batched outT flush."""
            h, half, on, i = f
            m, po = h // 2, (h % 2) * 64
            pool = tails["pool"] if tail else smps
            if tail:
                # tail is latency-bound: flush each transpose immediately,
                # on ACT (DVE carries the tail norm + y-copy stream)
                pst = pool.tile([128, 128], BF16, tag="tt", name="smt",
                                bufs=1)
                nc.tensor.transpose(pst[0:64, 0:128], on[:], identb[:])
                ig = half * 10 + i
                nc.vector.tensor_copy(
                    outT[m][po:po + 64, ig * 128:(ig + 1) * 128],
                    pst[0:64, 0:128])
                return
            # pair-batched transposes: even i allocates a [64,256] bf16 psum
            # tile, odd i completes it and flushes both to outT in one copy.
            if i % 2 == 0:
                tstate["grp"] = pool.tile([128, 256], BF16, tag="sm", name="smt")
            pst = tstate["grp"]
            nc.tensor.transpose(pst[0:64, (i % 2) * 128:(i % 2) * 128 + 128],
                                on[:], identb[:])
            if i % 2 == 1:
                ig0 = half * 10 + i - 1
                nc.vector.tensor_copy(
                    outT[m][po:po + 64, ig0 * 128:(ig0 + 2) * 128],
                    pst[0:64, 0:256])

        def y_step(i, tail=False):
            """y[i-tile] = outT[:, i].T @ Wo -> DRAM."""
            psy = (tails["pool"].tile([128, QD], F32, tag="ty", name="smy",
                                       bufs=3)
                   if tail else smps.tile([128, QD], F32, tag="sm", name="smy"))
            for kk in range(2):
                nc.tensor.matmul(
                    psy[:],
                    lhsT=outT[kk][:, i * 128:(i + 1) * 128],
                    rhs=wos[kk][:],
                    start=(kk == 0), stop=(kk == 1),
                )
            ysb = ypool.tile([128, QD], F32, tag="y", name="ysb")
            nc.scalar.copy(ysb[:], psy[:])
            nc.sync.dma_start(y[i * 128:i * 128 + 64, :], ysb[0:64, :])
            nc.gpsimd.dma_start(y[i * 128 + 64:(i + 1) * 128, :], ysb[64:128, :])

        # ---- emission: minimal upfront proj, rest interleaved ----
        # k cols 0:128 + q chunk A unblock the first scores matmul ASAP
        qk_proj(1, 0, 0, chunks=[(0, 128)])
        qk_proj(0, 0, 0)
        qk_proj(1, 0, 0, chunks=[(128, 384)])
        # remaining projection slabs, one CHUNK per slot; slots chosen on
        # even j that do NOT carry a pair-tile allocation (j % 4 == 2 does)
        # so the smps "sm" slot chain never stalls the in-order PE queue
        pend_list = [
            ((0, 0, 0), (1, 0, 0), 4), ((0, 0, 2), (1, 0, 0), 5),
            ((0, 0, 4), (1, 0, 1), 0), ((0, 0, 6), (1, 0, 1), 1),
            ((0, 0, 8), (1, 0, 1), 2), ((0, 0, 10), (0, 0, 1), 0),
            ((0, 0, 12), (0, 0, 1), 1), ((0, 0, 14), (0, 0, 1), 2),
            ((0, 1, 4), (0, 1, 0), 0), ((0, 1, 8), (0, 1, 0), 1),
            ((0, 1, 12), (0, 1, 0), 2), ((0, 1, 16), (1, 1, 0), 0),
            ((1, 0, 0), (1, 1, 0), 1), ((1, 0, 4), (1, 1, 0), 2),
            ((1, 0, 8), (1, 1, 1), 0), ((1, 0, 12), (1, 1, 1), 1),
            ((1, 0, 16), (1, 1, 1), 2), ((1, 1, 0), (0, 1, 1), 0),
            ((1, 1, 4), (0, 1, 1), 1), ((1, 1, 8), (0, 1, 1), 2),
        ]
        pending = {slot: (slab, ci) for slot, slab, ci in pend_list}

        fin_q = []
        prev = None
        for h in range(HPC):
            for half in range(2):
                ets = []
                for j in range(NT):
                    # ready PE work first so the spool-gated scores matmuls
                    # sit last in the in-order PE queue
                    if j % 2 == 0 and fin_q:
                        # (y-steps are all deferred to the tail so the (3,1)
                        # block's ACT stream stays exp-only.)
                        attn_fin(fin_q.pop(0))
                    pr = pending.pop((h, half, j), None)
                    if pr is not None:
                        (ti, m, ph_), ci = pr
                        allc = CHUNKS + [(0, 128), (512, 512), (1024, 256)]
                        qk_proj(ti, m, ph_, chunks=allc[ci:ci + 1])
                    if h == 0 and half == 0:
                        v_proj(j)
                    elif prev is not None and j % 2 == 1:
                        ph, phalf, pets = prev
                        i_mm = j // 2
                        pso = smps.tile([128, 65], F32, tag="sm", name="smo")
                        attn_mm_half(ph, phalf, pets, i_mm, 0, 14, pso)
                        tstate["pso"] = (ph, phalf, pets, pso, i_mm)
                    elif j % 2 == 0 and "pso" in tstate:
                        # complete the previous i's burst (carries the block
                        # it belongs to; at j=0 this is two blocks back)
                        ph, phalf, pets, pso, i_mm = tstate.pop("pso")
                        attn_mm_half(ph, phalf, pets, i_mm, 14, NT, pso)
                        fin_q.append(attn_norm(ph, phalf, pso, i_mm))
                    ets.append((scores_part1(h, half, j),
                                scores_part2(h, half, j)))
                prev = (h, half, ets)
        spool_cm.__exit__(None, None, None)
        tpool = ctx.enter_context(tc.tile_pool(name="tpool", bufs=2, space="PSUM"))
        tails["pool"] = tpool

        def drain_fin(tail):
            f = fin_q.pop(0)
            attn_fin(f, tail=tail)
            if f[0] == 3 and f[1] == 1:
                y_step(10 + f[3], tail=True)

        # complete the half-finished (3,0) i=9 burst, then leftover fins
        if "pso" in tstate:
            ph, phalf, pets, pso, i_mm = tstate.pop("pso")
            attn_mm_half(ph, phalf, pets, i_mm, 14, NT, pso)
            fin_q.append(attn_norm(ph, phalf, pso, i_mm))
        while fin_q:
            drain_fin(False)
        # tail: (3,1) attn pipelined with BOTH y half-streams. i>=6 attn
        # reads only the DVE-produced etb tiles (ready before ACT's last
        # exps), and the deferred first-half y's are ready immediately, so
        # both fill the queue while ACT drains its exp backlog.
        yq = list(range(10))
        for idx, i in enumerate([6, 7, 8, 9, 0, 1, 2, 3, 4, 5]):
            fin_q.append(attn_mm(3, 1, prev[2], i, tail=True))
            if yq:
                y_step(yq.pop(0), tail=True)
            if idx >= 1:
                drain_fin(True)
        while fin_q:
            drain_fin(True)

    nc.compile()
    return nc


def _get_nc():
    if "nc" not in _built:
        _built["nc"] = _build()
    return _built["nc"]


def kernel(x, Wq, Wk, Wv, Wo, bo):
    global last_results
    import ml_dtypes
    x = np.asarray(x, dtype=np.float32)
    Wq = np.asarray(Wq, dtype=np.float32)
    Wk = np.asarray(Wk, dtype=np.float32)
    Wv = np.asarray(Wv, dtype=np.float32)
    Wo = np.asarray(Wo, dtype=np.float32)
    bo = np.asarray(bo, dtype=np.float32)

    nc = _get_nc()
    in_maps = []
    for c in range(8):
        bb, g = divmod(c, 2)
        sl = slice(g * IPC, (g + 1) * IPC)
        in_maps.append({
            "xT": np.ascontiguousarray(x[bb].T).astype(ml_dtypes.bfloat16),
            "wq": np.ascontiguousarray(Wq[:, sl]).astype(ml_dtypes.bfloat16),
            "wk": np.ascontiguousarray(Wk[:, sl]).astype(ml_dtypes.bfloat16),
            "wv": np.ascontiguousarray(Wv[:, sl]).astype(ml_dtypes.bfloat16),
            "wo": np.ascontiguousarray(Wo[sl, :]).astype(ml_dtypes.bfloat16),
        })
    res = run_bass_kernel_spmd(nc, in_maps, core_ids=list(range(8)))
    last_results = res
    parts = [r["y"] for r in res.results]
    out = np.empty((B, N, QD), dtype=np.float32)
    for bb in range(B):
        out[bb] = parts[2 * bb] + parts[2 * bb + 1]
    out += bo
    return out


# revision 9
# speedup vs baseline: 1.0570x; 1.0039x over previous
"""Fused multi-head self-attention kernel for Trainium2 (Bass/Tile). v2.

Problem: x:[4,2560,320] f32, Wq/Wk/Wv:[320,512], Wo:[512,320], bo:[320]
  q,k,v = x@W*, 8 heads x 64; sim = q k^T * d^-0.5; attn = softmax(sim);
  out = (attn @ v) @ Wo + bo.

Sharding: batch*head 32-way -> 8 cores: core c handles batch c//2 and the
4-head group c%2. Host sums the two half-head partial output projections
per batch and adds the bias.

Per-core layout trick: scores are computed TRANSPOSED (sT[j,i] = k_j . q_i)
so that the softmax denominator arrives for free: v gets a ones-column
appended, and out' = expT_slice.T @ [v|1] accumulates both attn@v and the
row sums. Normalisation is a per-partition reciprocal+multiply.

Engine budget (cost model, per core): PE ~160us is the floor (scores
204.8k cyc + attn@v 104k + projections/transposes ~69k). The ACT-engine
exp over 26M elements (~218us alone -- the old bottleneck) is split:
 - ACT computes exp on score cols [0:ACOLS] natively; DVE computes cols
   [ACOLS:1280] with a one-instruction Schraudolph bitcast exp
   (int16(x*A+B) viewed as bf16; ~1.8% rms there, 40% of weights).
 - eta/etb live in SEPARATE tiles and the scores land in two psum tiles
   (s1 ACT-read / s2 DVE-read), so neither engine's slot chain gates the
   other (shared tiles serialize via write-write/read hazards).
 - x/Wq/Wk/Wv/Wo are bf16 (host-side cast): halves DMA + SBUF footprint.
 - adjacent heads (2m, 2m+1) normalize into ONE shared [128,128] tile
   (complementary outT partition ranges), so a single PE transpose + one
   full-column flush serves the whole head pair; the transpose+flush is
   deferred so the in-order PE queue never waits on the DVE normalize.
 - all y-steps run in the tail (keeps the last block's ACT stream
   exp-only); tail attn runs i>=6 first (needs only DVE-produced etb).
"""

import sys

import numpy as np

if "/opt/trn_rl_repo" not in sys.path:
    sys.path.insert(0, "/opt/trn_rl_repo")

from contextlib import ExitStack

import concourse.bass as bass
from concourse import bacc
import concourse.mybir as mybir
import concourse.tile as tile
from concourse.bass_utils import run_bass_kernel_spmd
from concourse.masks import make_identity

# ---- problem constants (hardcoded per contract) ----
B = 4
N = 2560
QD = 320
H_TOT = 8
D = 64
HPC = 4                  # heads per core
IPC = HPC * D            # 256 inner dims per core
SCALE = D ** -0.5
NT = N // 128            # 20 n-tiles
HALF = N // 2            # 1280
F32 = mybir.dt.float32
BF16 = mybir.dt.bfloat16
F32R = mybir.dt.float32r
I16 = mybir.dt.int16
EXP = mybir.ActivationFunctionType.Exp

# qd (=320) split into K subtiles for the 128-partition contraction
KS = [(0, 128), (128, 128), (256, 64)]
# 1280-wide column chunks (PSUM-bank-aligned matmul N<=512)
CHUNKS = [(0, 512), (512, 512), (1024, 256)]

# exp split: ACT native exp on [0:ACOLS], DVE Schraudolph on [ACOLS:HALF].
# ACOLS must be a multiple of 128 so the two exp outputs can live in
# SEPARATE tiles (shared-tile writes serialize ACT behind DVE via a
# write-write hazard) with attnv i-tiles reading wholly from one of them.
ACOLS = 768
A_MUL = 128.0 * SCALE / np.log(2.0)       # folds the 1/sqrt(d) scale
B_ADD = 128.0 * (127.0 - 0.058)           # rounding-convert-optimal bias

EXP_BUFS = 42            # expT pool slots of [128,1280] bf16 (2.5KB/part each)

_built = {}
last_results = None      # stashed BassKernelResults for the test harness


def _build():
    nc = bacc.Bacc(None, target_bir_lowering=False)
    xT = nc.declare_dram_parameter("xT", [QD, N], BF16, isOutput=False)
    wq = nc.declare_dram_parameter("wq", [QD, IPC], BF16, isOutput=False)
    wk = nc.declare_dram_parameter("wk", [QD, IPC], BF16, isOutput=False)
    wv = nc.declare_dram_parameter("wv", [QD, IPC], BF16, isOutput=False)
    wo = nc.declare_dram_parameter("wo", [IPC, QD], BF16, isOutput=False)
    y = nc.declare_dram_parameter("y", [N, QD], F32, isOutput=True)

    with tile.TileContext(nc) as tc, ExitStack() as ctx:
        const = ctx.enter_context(tc.tile_pool(name="const", bufs=1))
        smps = ctx.enter_context(tc.tile_pool(name="smps", bufs=2, space="PSUM"))
        epool = ctx.enter_context(tc.tile_pool(name="epool", bufs=EXP_BUFS))
        sbsm = ctx.enter_context(tc.tile_pool(name="sbsm", bufs=4))
        ypool = ctx.enter_context(tc.tile_pool(name="ypool", bufs=4))
        onpool = ctx.enter_context(tc.tile_pool(name="onpool", bufs=1))
        spool_cm = tc.tile_pool(name="spool", bufs=2, space="PSUM")
        spool = spool_cm.__enter__()

        ident = const.tile([128, 128], F32, tag="ident", name="ident")
        make_identity(nc, ident[:])
        identb = const.tile([128, 128], BF16, tag="identb", name="identb")
        nc.vector.tensor_copy(identb[:], ident[:])
        warm = sbsm.tile([128, 1], F32, tag="rc", name="warm")
        nc.scalar.activation(warm[:], ident[:, 0:1], EXP, scale=1.0)
        for _ in range(4):
            pw = smps.tile([128, 128], F32, tag="sm", name="pwarm")
            nc.tensor.matmul(pw[:], lhsT=ident[:], rhs=ident[:],
                             start=True, stop=True)

        # ---- persistent inputs (DMA emission ordered by first use) ----
        xts = [const.tile([128, N], BF16, tag=f"xt{ki}", name=f"xt{ki}")
               for ki in range(3)]
        wqs = [const.tile([128, IPC], BF16, tag=f"wq{ki}", name=f"wq{ki}")
               for ki in range(3)]
        wks = [const.tile([128, IPC], BF16, tag=f"wk{ki}", name=f"wk{ki}")
               for ki in range(3)]
        wvs = [const.tile([128, IPC], BF16, tag=f"wv{ki}", name=f"wv{ki}")
               for ki in range(3)]
        wos = [const.tile([128, QD], BF16, tag=f"wo{kk}", name=f"wo{kk}")
               for kk in range(2)]
        # critical set first; x split across BOTH dma queues (sync + gpsimd)
        for ki, (k0, kw) in enumerate(KS):
            nc.sync.dma_start(xts[ki][:kw, 0:640], xT[k0:k0 + kw, 0:640])
            nc.gpsimd.dma_start(wqs[ki][:kw, :], wq[k0:k0 + kw, :])
        for ki, (k0, kw) in enumerate(KS):
            nc.sync.dma_start(xts[ki][:kw, 640:1280], xT[k0:k0 + kw, 640:1280])
            nc.gpsimd.dma_start(wks[ki][:kw, :], wk[k0:k0 + kw, :])
        for ki, (k0, kw) in enumerate(KS):
            nc.gpsimd.dma_start(xts[ki][:kw, 1280:1920], xT[k0:k0 + kw, 1280:1920])
            nc.sync.dma_start(xts[ki][:kw, 1920:2560], xT[k0:k0 + kw, 1920:2560])
        for ki, (k0, kw) in enumerate(KS):
            nc.gpsimd.dma_start(wvs[ki][:kw, :], wv[k0:k0 + kw, :])
        for kk in range(2):
            nc.gpsimd.dma_start(wos[kk][:], wo[kk * 128:(kk + 1) * 128, :])

        # qT/kT: [inner(256) x n] as 2 tiles of [128, N] each; fp32 storage
        qk_sb = [const.tile([128, N], F32R, tag=f"qk{i}", name=f"qk{i}") for i in range(4)]
        # outT: normalized attention output, [inner x n], bf16
        outT = [const.tile([128, N], BF16, tag=f"oT{kk}", name=f"oT{kk}") for kk in range(2)]
        # v with ones column per head: [n-tile][128, 4*65] bf16
        v1s = [const.tile([128, HPC * 65], BF16, tag=f"v1_{j}", name=f"v1_{j}") for j in range(NT)]

        ws = [wqs, wks]
        tails = {}
        tstate = {}

        def qk_proj(ti, m, half, chunks=None):
            """qT/kT tile ti(0=q,1=k), inner slab m, col half -> qk_sb[ti*2+m].

            PSUM->SBUF copies: 512-wide chunks go to ACT (has startup slack),
            the 256-wide chunk to DVE, to keep early-block DVE load down.
            """
            for c0, cw in (chunks or CHUNKS):
                ps = smps.tile([128, 512], F32, tag="sm", name="smp")
                for ki, (k0, kw) in enumerate(KS):
                    nc.tensor.matmul(
                        ps[:, 0:cw],
                        lhsT=ws[ti][ki][:kw, m * 128:(m + 1) * 128],
                        rhs=xts[ki][:kw, half * HALF + c0:half * HALF + c0 + cw],
                        start=(ki == 0), stop=(ki == 2),
                    )
                dst = qk_sb[ti * 2 + m][:, half * HALF + c0:half * HALF + c0 + cw]
                nc.vector.tensor_copy(dst, ps[:, 0:cw])

        def v_proj(j):
            """v for n-tile j (all 4 heads) -> v1s[j] bf16 with ones cols."""
            ps = smps.tile([128, IPC], F32, tag="sm", name="smv")
            for ki, (k0, kw) in enumerate(KS):
                nc.tensor.matmul(
                    ps[:],
                    lhsT=xts[ki][:kw, j * 128:(j + 1) * 128],
                    rhs=wvs[ki][:kw, :],
                    start=(ki == 0), stop=(ki == 2),
                )
            v1v = v1s[j][:].rearrange("p (h e) -> p h e", e=65)
            nc.gpsimd.memset(v1v[:, :, 64:65], 1.0)
            # alternate ACT/DVE so neither engine eats all 20 copies while
            # also chewing the first block's exp stream
            src = ps[:].rearrange("p (h d) -> p h d", d=64)
            if j % 2 == 0:
                nc.vector.tensor_copy(v1v[:, :, 0:64], src)
            else:
                nc.scalar.copy(v1v[:, :, 0:64], src)

        def scores_part1(h, half, j):
            """s1 half of the scores (cols 0:ACOLS) + the ACT exp -> eta.

            s1 is read only by ACT, s2 only by DVE, so each engine gates only
            its own psum slot chain. ACT is the steady-state pacer, so part1
            is emitted FIRST in each j-body to keep its exps back-to-back.
            """
            m, po = h // 2, (h % 2) * 64
            ps1 = spool.tile([128, ACOLS], F32, tag="s1", name="s1")
            for c0, cw in ((0, 512), (512, 256)):
                nc.tensor.matmul(
                    ps1[:, c0:c0 + cw],
                    lhsT=qk_sb[2 + m][po:po + 64, j * 128:(j + 1) * 128],
                    rhs=qk_sb[m][po:po + 64, half * HALF + c0:half * HALF + c0 + cw],
                    start=True, stop=True,
                )
            eta = epool.tile([128, ACOLS], BF16, tag="ea", name="eta")
            nc.scalar.activation(eta[:], ps1[:], EXP, scale=float(SCALE))
            return eta

        def scores_part2(h, half, j):
            """s2 half of the scores (cols ACOLS:) + the DVE exp -> etb."""
            m, po = h // 2, (h % 2) * 64
            ps2 = spool.tile([128, HALF - ACOLS], F32, tag="s2", name="s2")
            nc.tensor.matmul(
                ps2[:],
                lhsT=qk_sb[2 + m][po:po + 64, j * 128:(j + 1) * 128],
                rhs=qk_sb[m][po:po + 64, half * HALF + ACOLS:(half + 1) * HALF],
                start=True, stop=True,
            )
            etb = epool.tile([128, HALF - ACOLS], BF16, tag="eb", name="etb")
            nc.vector.tensor_scalar(
                etb[:].bitcast(I16), ps2[:],
                float(A_MUL), float(B_ADD),
                mybir.AluOpType.mult, mybir.AluOpType.add)
            return etb

        def attn_mm_half(h, half, ets, i, jlo, jhi, pso, tail=False):
            """Half of out'[i-tile] = sum_j expT_j[:, i].T @ [v|1]."""
            na = ACOLS // 128
            for j in range(jlo, jhi):
                eta, etb = ets[j]
                lhsT = (eta[:, i * 128:(i + 1) * 128] if i < na
                        else etb[:, (i - na) * 128:(i - na + 1) * 128])
                nc.tensor.matmul(
                    pso[:],
                    lhsT=lhsT,
                    rhs=v1s[j][:, h * 65:(h + 1) * 65],
                    start=(j == jlo and jlo == 0), stop=(j == NT - 1),
                )

        onpair = {}

        def attn_norm(h, half, pso, i):
            """Normalize into half of a head-pair tile: heads 2m and 2m+1
            share one [128,128] tile (cols 0:64 / 64:128) so ONE transpose
            later flips both into outT's complementary partition ranges."""
            rc = sbsm.tile([128, 1], F32, tag="rc", name="rc")
            nc.vector.reciprocal(rc[:], pso[:, 64:65])
            if h % 2 == 0:
                onp = onpool.tile([128, 128], BF16, tag=f"o{half}_{i}",
                                  name="onp")
                onpair[(half, i)] = onp
            else:
                onp = onpair[(half, i)]
            po = (h % 2) * 64
            nc.vector.tensor_scalar_mul(onp[:, po:po + 64], pso[:, 0:64], rc[:])
            return (h, half, onp, i)

        def attn_mm(h, half, ets, i, tail=False):
            """out'[i-tile] = sum_j expT_j[:, i].T @ [v|1]; normalize to 'on'.

            The transpose+flush is deferred (attn_fin) so the in-order PE
            queue never waits on the DVE normalize of the same step.
            """
            pool = tails["pool"] if tail else smps
            pso = pool.tile([128, 65], F32, tag="to" if tail else "sm", name="smo")
            attn_mm_half(h, half, ets, i, 0, NT, pso, tail)
            return attn_norm(h, half, pso, i)

        def attn_fin(f, tail=False):
            """One [128,128] PE transpose per HEAD-PAIR + full-column flush.

            Even heads only fill their half of the pair tile (no PE work);
            odd heads transpose the completed pair and flush both heads'
            outT partition ranges in one copy."""
            h, half, onp, i = f
            if h % 2 == 0:
                return
            m = h // 2
            pool = tails["pool"] if tail else smps
            pst = pool.tile([128, 128], BF16, tag="tt" if tail else "sm",
                            name="smt", bufs=1 if tail else None)
            nc.tensor.transpose(pst[:], onp[:], identb[:])
            ig = half * 10 + i
            nc.vector.tensor_copy(
                outT[m][:, ig * 128:(ig + 1) * 128], pst[:])

        def y_step(i, tail=False):
            """y[i-tile] = outT[:, i].T @ Wo -> DRAM."""
            psy = (tails["pool"].tile([128, QD], F32, tag="ty", name="smy",
                                       bufs=3)
                   if tail else smps.tile([128, QD], F32, tag="sm", name="smy"))
            for kk in range(2):
                nc.tensor.matmul(
                    psy[:],
                    lhsT=outT[kk][:, i * 128:(i + 1) * 128],
                    rhs=wos[kk][:],
                    start=(kk == 0), stop=(kk == 1),
                )
            ysb = ypool.tile([128, QD], F32, tag="y", name="ysb")
            nc.scalar.copy(ysb[:], psy[:])
            nc.sync.dma_start(y[i * 128:i * 128 + 64, :], ysb[0:64, :])
            nc.gpsimd.dma_start(y[i * 128 + 64:(i + 1) * 128, :], ysb[64:128, :])

        # ---- emission: minimal upfront proj, rest interleaved ----
        # k cols 0:128 + q chunk A unblock the first scores matmul ASAP
        qk_proj(1, 0, 0, chunks=[(0, 128)])
        qk_proj(0, 0, 0)
        qk_proj(1, 0, 0, chunks=[(128, 384)])
        # remaining projection slabs, one CHUNK per slot; slots chosen on
        # even j that do NOT carry a pair-tile allocation (j % 4 == 2 does)
        # so the smps "sm" slot chain never stalls the in-order PE queue
        pend_list = [
            ((0, 0, 0), (1, 0, 0), 4), ((0, 0, 2), (1, 0, 0), 5),
            ((0, 0, 4), (1, 0, 1), 0), ((0, 0, 6), (1, 0, 1), 1),
            ((0, 0, 8), (1, 0, 1), 2), ((0, 0, 10), (0, 0, 1), 0),
            ((0, 0, 12), (0, 0, 1), 1), ((0, 0, 14), (0, 0, 1), 2),
            ((0, 1, 4), (0, 1, 0), 0), ((0, 1, 8), (0, 1, 0), 1),
            ((0, 1, 12), (0, 1, 0), 2), ((0, 1, 16), (1, 1, 0), 0),
            ((1, 0, 0), (1, 1, 0), 1), ((1, 0, 4), (1, 1, 0), 2),
            ((1, 0, 8), (1, 1, 1), 0), ((1, 0, 12), (1, 1, 1), 1),
            ((1, 0, 16), (1, 1, 1), 2), ((1, 1, 0), (0, 1, 1), 0),
            ((1, 1, 4), (0, 1, 1), 1), ((1, 1, 8), (0, 1, 1), 2),
        ]
        pending = {slot: (slab, ci) for slot, slab, ci in pend_list}

        fin_q = []
        prev = None
        for h in range(HPC):
            for half in range(2):
                ets = []
                for j in range(NT):
                    # ready PE work first so the spool-gated scores matmuls
                    # sit last in the in-order PE queue
                    if j % 2 == 0 and fin_q:
                        # (y-steps are all deferred to the tail so the (3,1)
                        # block's ACT stream stays exp-only.)
                        attn_fin(fin_q.pop(0))
                    pr = pending.pop((h, half, j), None)
                    if pr is not None:
                        (ti, m, ph_), ci = pr
                        allc = CHUNKS + [(0, 128), (512, 512), (1024, 256)]
                        qk_proj(ti, m, ph_, chunks=allc[ci:ci + 1])
                    if h == 0 and half == 0:
                        v_proj(j)
                    elif prev is not None and j % 2 == 1:
                        ph, phalf, pets = prev
                        i_mm = j // 2
                        pso = smps.tile([128, 65], F32, tag="sm", name="smo")
                        attn_mm_half(ph, phalf, pets, i_mm, 0, 13, pso)
                        tstate["pso"] = (ph, phalf, pets, pso, i_mm)
                    elif j % 2 == 0 and "pso" in tstate:
                        # complete the previous i's burst (carries the block
                        # it belongs to; at j=0 this is two blocks back)
                        ph, phalf, pets, pso, i_mm = tstate.pop("pso")
                        attn_mm_half(ph, phalf, pets, i_mm, 13, NT, pso)
                        fin_q.append(attn_norm(ph, phalf, pso, i_mm))
                    ets.append((scores_part1(h, half, j),
                                scores_part2(h, half, j)))
                prev = (h, half, ets)
        spool_cm.__exit__(None, None, None)
        tpool = ctx.enter_context(tc.tile_pool(name="tpool", bufs=2, space="PSUM"))
        tails["pool"] = tpool

        def drain_fin(tail):
            f = fin_q.pop(0)
            attn_fin(f, tail=tail)
            if f[0] == 3 and f[1] == 1:
                y_step(10 + f[3], tail=True)

        # complete the half-finished (3,0) i=9 burst, then leftover fins
        if "pso" in tstate:
            ph, phalf, pets, pso, i_mm = tstate.pop("pso")
            attn_mm_half(ph, phalf, pets, i_mm, 13, NT, pso)
            fin_q.append(attn_norm(ph, phalf, pso, i_mm))
        while fin_q:
            drain_fin(False)
        # tail: (3,1) attn pipelined with BOTH y half-streams. i>=6 attn
        # reads only the DVE-produced etb tiles (ready before ACT's last
        # exps), and the deferred first-half y's are ready immediately, so
        # both fill the queue while ACT drains its exp backlog.
        yq = list(range(10))
        for idx, i in enumerate([6, 7, 8, 9, 0, 1, 2, 3, 4, 5]):
            fin_q.append(attn_mm(3, 1, prev[2], i, tail=True))
            if yq:
                y_step(yq.pop(0), tail=True)
            if idx >= 1:
                drain_fin(True)
        while fin_q:
            drain_fin(True)

    nc.compile()
    return nc


def _get_nc():
    if "nc" not in _built:
        _built["nc"] = _build()
    return _built["nc"]


def kernel(x, Wq, Wk, Wv, Wo, bo):
    global last_results
    import ml_dtypes
    x = np.asarray(x, dtype=np.float32)
    Wq = np.asarray(Wq, dtype=np.float32)
    Wk = np.asarray(Wk, dtype=np.float32)
    Wv = np.asarray(Wv, dtype=np.float32)
    Wo = np.asarray(Wo, dtype=np.float32)
    bo = np.asarray(bo, dtype=np.float32)

    nc = _get_nc()
    in_maps = []
    for c in range(8):
        bb, g = divmod(c, 2)
        sl = slice(g * IPC, (g + 1) * IPC)
        in_maps.append({
            "xT": np.ascontiguousarray(x[bb].T).astype(ml_dtypes.bfloat16),
            "wq": np.ascontiguousarray(Wq[:, sl]).astype(ml_dtypes.bfloat16),
            "wk": np.ascontiguousarray(Wk[:, sl]).astype(ml_dtypes.bfloat16),
            "wv": np.ascontiguousarray(Wv[:, sl]).astype(ml_dtypes.bfloat16),
            "wo": np.ascontiguousarray(Wo[sl, :]).astype(ml_dtypes.bfloat16),
        })
    res = run_bass_kernel_spmd(nc, in_maps, core_ids=list(range(8)))
    last_results = res
    parts = [r["y"] for r in res.results]
    out = np.empty((B, N, QD), dtype=np.float32)
    for bb in range(B):
        out[bb] = parts[2 * bb] + parts[2 * bb + 1]
    out += bo
    return out
